# revision 1
# baseline (speedup 1.0000x reference)
"""HGT (2-type, 2-relation, 2-layer) Bass kernel for 8 Trainium2 cores.

Sharding: destination-node sharding. Core c owns dst rows [5120c, 5120(c+1))
of both node types (core 7 partially padded). Each core builds the full
folded K'/V' source tables (projections replicated), gathers per-edge
source rows with dma_gather (int16 indices; src space split at 32768 into
lo/hi sub-tables), computes per-edge attention with one-hot matmuls per
128-edge block (segment softmax without max-subtraction), and accumulates
per 128-dst group in PSUM. New node features are AllGathered between the
two layers.
"""
import math
import os
import sys

import numpy as np

sys.path.insert(0, "/opt/trn_rl_repo")

H, D, C, L = 4, 32, 128, 2
INV_SQRT_D = 1.0 / math.sqrt(D)
P = 128
NCORES = 8
SHARD = 5120          # 40 groups of 128 dst rows per core
NGRP = SHARD // P     # 40
NPAD = NCORES * SHARD # 40960
LO_LIM = 32768
HI_ROWS = NPAD - LO_LIM
CHUNK_BLK = 12        # gather chunk size in 128-edge blocks

LAST_RESULT = None


def _fold_weights(ins):
    """Fold a_rel/m_rel into k/v weights, p_rel/sqrt(D) into q, sigmoid(skip)
    into a_lin. Returns dict of numpy arrays."""
    f = {}
    for l in range(L):
        for t in range(2):
            kw = np.asarray(ins["k_w"][l, t])   # [C, C]
            kb = np.asarray(ins["k_b"][l, t])   # [C]
            vw = np.asarray(ins["v_w"][l, t])
            vb = np.asarray(ins["v_b"][l, t])
            ar = np.asarray(ins["a_rel"][l, t])  # [H, D, D]; type t is src of rel t
            mr = np.asarray(ins["m_rel"][l, t])
            wk = np.zeros((C, C), np.float32)
            wv = np.zeros((C, C), np.float32)
            bk = np.zeros(C, np.float32)
            bv = np.zeros(C, np.float32)
            for h in range(H):
                sl = slice(h * D, (h + 1) * D)
                wk[:, sl] = kw[:, sl] @ ar[h]
                wv[:, sl] = vw[:, sl] @ mr[h]
                bk[sl] = kb[sl] @ ar[h]
                bv[sl] = vb[sl] @ mr[h]
            f[f"Wkv{l}{t}"] = np.concatenate([wk, wv], axis=1)          # [C, 2C]
            f[f"Bkv{l}{t}"] = np.tile(np.concatenate([bk, bv])[None, :], (P, 1))
            # q: type t is dst of relation (1 - t)
            r_dst = 1 - t
            pr = np.asarray(ins["p_rel"][l, r_dst]) * INV_SQRT_D        # [H]
            scale = np.repeat(pr, D)                                    # [C]
            f[f"Wq{l}{t}"] = (np.asarray(ins["q_w"][l, t]) * scale[None, :]).astype(np.float32)
            f[f"Bq{l}{t}"] = np.tile((np.asarray(ins["q_b"][l, t]) * scale)[None, :], (P, 1))
            s = 1.0 / (1.0 + math.exp(-float(np.asarray(ins["skip"][l, t]))))
            f[f"Wal{l}{t}"] = (np.asarray(ins["a_lin_w"][l, t]) * s).astype(np.float32)
            f[f"Bal{l}{t}"] = np.tile((np.asarray(ins["a_lin_b"][l, t]) * s)[None, :], (P, 1))
            f[f"oms{l}{t}"] = 1.0 - s
    f["Wina"] = np.asarray(ins["lin_a_w"]).astype(np.float32)
    f["Binb"] = np.tile(np.asarray(ins["lin_b_b"])[None, :], (P, 1)).astype(np.float32)
    f["Bina"] = np.tile(np.asarray(ins["lin_a_b"])[None, :], (P, 1)).astype(np.float32)
    f["Winb"] = np.asarray(ins["lin_b_w"]).astype(np.float32)
    return f


def _prep_edges(edge):
    """Partition one relation's edges by dst shard; build per-core gather
    index / dst_rel arrays plus the shared static block schedule.

    Returns (idx_w[8], dst_rel_t[8], nblk_lo[NGRP], nblk_hi[NGRP])."""
    src = np.asarray(edge[0]).astype(np.int64)
    dst = np.asarray(edge[1]).astype(np.int64)
    core = dst // SHARD
    per = []  # per core: list over groups of (lo_src, lo_rel, hi_src, hi_rel)
    nblk_lo = np.zeros(NGRP, np.int64)
    nblk_hi = np.zeros(NGRP, np.int64)
    for c in range(NCORES):
        m = core == c
        s, dl = src[m], dst[m] - c * SHARD
        g = dl // P
        rel = dl % P
        glists = []
        for gi in range(NGRP):
            gm = g == gi
            sg, rg = s[gm], rel[gm]
            lo = sg < LO_LIM
            glists.append((sg[lo], rg[lo], sg[~lo] - LO_LIM, rg[~lo]))
            nblk_lo[gi] = max(nblk_lo[gi], (len(sg[lo]) + P - 1) // P)
            nblk_hi[gi] = max(nblk_hi[gi], (len(sg[~lo]) + P - 1) // P)
    nblk_lo = np.maximum(nblk_lo, 1)
    TB = int(nblk_lo.sum() + nblk_hi.sum())
    SL = TB * P
    idx_ws, dr_ts = [], []
    for c in range(NCORES):
        m = core == c
        s, dl = src[m], dst[m] - c * SHARD
        g = dl // P
        rel = dl % P
        idx = np.zeros(SL, np.int16)
        dr = np.full(SL, -1.0, np.float32)
        pos = 0
        for gi in range(NGRP):
            gm = g == gi
            sg, rg = s[gm], rel[gm]
            lo = sg < LO_LIM
            sl_, rl_ = sg[lo], rg[lo]
            idx[pos:pos + len(sl_)] = sl_
            dr[pos:pos + len(sl_)] = rl_
            pos += int(nblk_lo[gi]) * P
        for gi in range(NGRP):
            gm = g == gi
            sg, rg = s[gm], rel[gm]
            hi = sg >= LO_LIM
            sh_, rh_ = sg[hi] - LO_LIM, rg[hi]
            idx[pos:pos + len(sh_)] = sh_
            dr[pos:pos + len(sh_)] = rh_
            pos += int(nblk_hi[gi]) * P
        assert pos == SL
        idx_ws.append(np.tile(idx.reshape(SL // 16, 16).T, (8, 1)).copy())
        dr_ts.append(dr.reshape(TB, P).T.copy())
    return idx_ws, dr_ts, nblk_lo.tolist(), nblk_hi.tolist()


def _chunks(nblk_per_grp):
    """Split the per-group block list of one region into gather chunks of up
    to CHUNK_BLK blocks. Returns (chunk list [(blk_start, nblk)],
    per-group [(chunk_i, local_b)])."""
    total = sum(nblk_per_grp)
    chunks = []
    b = 0
    while b < total:
        n = min(CHUNK_BLK, total - b)
        chunks.append((b, n))
        b += n
    loc = []
    b = 0
    for g, n in enumerate(nblk_per_grp):
        lst = []
        for i in range(n):
            blk = b + i
            ci = blk // CHUNK_BLK
            lst.append((ci, blk - chunks[ci][0]))
        loc.append(lst)
        b += n
    return chunks, loc


def kernel(**ins):
    global LAST_RESULT
    import concourse.bass as bass
    import concourse.tile as tile
    from concourse import bacc, mybir
    from concourse.bass_utils import run_bass_kernel_spmd
    from concourse.masks import make_identity

    FP = mybir.dt.float32
    I16 = mybir.dt.int16
    I32 = mybir.dt.int32
    AL = mybir.AluOpType
    AF = mybir.ActivationFunctionType

    f = _fold_weights(ins)
    idx0, dr0, nlo0, nhi0 = _prep_edges(np.asarray(ins["edge_ab"]))
    idx1, dr1, nlo1, nhi1 = _prep_edges(np.asarray(ins["edge_ba"]))
    rel_meta = [(nlo0, nhi0), (nlo1, nhi1)]
    TBs = [sum(nlo0) + sum(nhi0), sum(nlo1) + sum(nhi1)]

    xa = np.asarray(ins["x_a"]).astype(np.float32)
    xb = np.asarray(ins["x_b"]).astype(np.float32)
    DA, DB = xa.shape[1], xb.shape[1]
    xaT = np.zeros((DA, NPAD), np.float32)
    xaT[:, :40000] = xa.T
    xbT = np.zeros((DB, NPAD), np.float32)
    xbT[:, :40000] = xb.T

    nc = bacc.Bacc("TRN2", target_bir_lowering=False, debug=False, num_devices=NCORES)

    # ---- DRAM tensors ----
    t_xaT = nc.dram_tensor("xaT", [DA, NPAD], FP, kind="ExternalInput").ap()
    t_xbT = nc.dram_tensor("xbT", [DB, NPAD], FP, kind="ExternalInput").ap()
    t_xasT = nc.dram_tensor("xasT", [DA, SHARD], FP, kind="ExternalInput").ap()
    t_xbsT = nc.dram_tensor("xbsT", [DB, SHARD], FP, kind="ExternalInput").ap()
    wnames = ["Wina", "Winb", "Bina", "Binb"]
    for l in range(L):
        for t in range(2):
            wnames += [f"Wkv{l}{t}", f"Bkv{l}{t}", f"Wq{l}{t}", f"Bq{l}{t}",
                       f"Wal{l}{t}", f"Bal{l}{t}"]
    t_w = {n: nc.dram_tensor(n, list(f[n].shape), FP, kind="ExternalInput").ap()
           for n in wnames}
    t_idx = [nc.dram_tensor(f"idx{r}", [P, TBs[r] * 8], I16, kind="ExternalInput").ap()
             for r in range(2)]
    t_dr = [nc.dram_tensor(f"dr{r}", [P, TBs[r]], FP, kind="ExternalInput").ap()
            for r in range(2)]

    t_kv = [nc.dram_tensor(f"kv{t}", [NPAD, 2 * C], FP) for t in range(2)]
    t_x0s = [nc.dram_tensor(f"x0s{t}", [SHARD, C], FP) for t in range(2)]
    t_x1s = [nc.dram_tensor(f"x1s{t}", [SHARD, C], FP) for t in range(2)]
    t_nxT = [nc.dram_tensor(f"nxT{t}", [P, SHARD], FP) for t in range(2)]
    t_ag = [nc.dram_tensor(f"ag{t}", [NCORES, P, SHARD], FP, addr_space="Shared")
            for t in range(2)]
    t_out = [nc.dram_tensor(f"out{t}", [SHARD, C], FP, kind="ExternalOutput").ap()
             for t in range(2)]

    with tile.TileContext(nc) as tc:
        cpool_cm = tc.tile_pool(name="const", bufs=1)
        cpool = cpool_cm.__enter__()
        ident = cpool.tile([P, P], FP)
        make_identity(nc, ident[:])
        ioi = cpool.tile([P, P], I32)
        nc.gpsimd.iota(ioi[:], pattern=[[1, P]], base=0, channel_multiplier=0)
        iota_row = cpool.tile([P, P], FP)
        nc.vector.tensor_copy(iota_row[:], ioi[:])
        ioc = cpool.tile([P, 1], I32)
        nc.gpsimd.iota(ioc[:], pattern=[[0, 1]], base=0, channel_multiplier=1)
        iota_col = cpool.tile([P, 1], FP)
        nc.vector.tensor_copy(iota_col[:], ioc[:])
        w_sb = {}
        for n in wnames:
            w_sb[n] = cpool.tile(list(f[n].shape), FP, name=n, tag=n)
            nc.sync.dma_start(out=w_sb[n][:], in_=t_w[n][:])
        dr_sb = []
        for r in range(2):
            drt = cpool.tile([P, TBs[r]], FP, name=f"drsb{r}", tag=f"drsb{r}")
            nc.sync.dma_start(out=drt[:], in_=t_dr[r][:])
            dr_sb.append(drt)
        idx_sb = []
        for r in range(2):
            it = cpool.tile([P, TBs[r] * 8], I16, name=f"idxsb{r}", tag=f"idxsb{r}")
            nc.sync.dma_start(out=it[:], in_=t_idx[r][:])
            idx_sb.append(it)
        q_sb = [cpool.tile([P, NGRP, C], FP, name=f"qsb{t}", tag=f"qsb{t}") for t in range(2)]
        acc_sb = cpool.tile([P, NGRP, 132], FP)

        # ---------- layer-0 full tables (two-stage projection) ----------
        def input_proj(xT_ap, Win, Bin, t, j, src_pool, ps_pool, out_pool):
            DIN = xT_ap.shape[0]
            lhs = src_pool.tile([DIN, P], FP, tag="lhs0")
            nc.sync.dma_start(out=lhs[:], in_=xT_ap[:, j * P:(j + 1) * P])
            ps1 = ps_pool.tile([P, C], FP, space="PSUM", tag="ps1")
            nc.tensor.matmul(out=ps1[:], lhsT=lhs[:], rhs=w_sb[Win][:], start=True, stop=True)
            x0 = out_pool.tile([P, C], FP, tag="x0")
            nc.vector.tensor_tensor(out=x0[:], in0=ps1[:], in1=w_sb[Bin][:], op=AL.add)
            x0r = out_pool.tile([P, C], FP, tag="x0r")
            nc.scalar.activation(out=x0r[:], in_=x0[:], func=AF.Relu)
            pst = ps_pool.tile([P, P], FP, space="PSUM", tag="pst0")
            nc.tensor.transpose(out=pst[:], in_=x0r[:], identity=ident[:])
            x0T = out_pool.tile([P, P], FP, tag="x0T")
            nc.vector.tensor_copy(x0T[:], pst[:])
            return x0r, x0T

        with (
            tc.tile_pool(name="p0src", bufs=3) as src_pool,
            tc.tile_pool(name="p0ps", bufs=2, space="PSUM") as ps_pool,
            tc.tile_pool(name="p0out", bufs=3) as out_pool,
        ):
            for t, (xT_ap, Win, Bin) in enumerate(
                [(t_xaT, "Wina", "Bina"), (t_xbT, "Winb", "Binb")]
            ):
                for j in range(NPAD // P):
                    _, x0T = input_proj(xT_ap, Win, Bin, t, j, src_pool, ps_pool, out_pool)
                    ps2 = ps_pool.tile([P, 2 * C], FP, space="PSUM", tag="ps2")
                    nc.tensor.matmul(out=ps2[:], lhsT=x0T[:], rhs=w_sb[f"Wkv0{t}"][:],
                                     start=True, stop=True)
                    kvt = out_pool.tile([P, 2 * C], FP, tag="kvt")
                    nc.vector.tensor_tensor(out=kvt[:], in0=ps2[:], in1=w_sb[f"Bkv0{t}"][:], op=AL.add)
                    nc.sync.dma_start(out=t_kv[t].ap()[j * P:(j + 1) * P, :], in_=kvt[:])
            # shard pass: x0 shard rm + q0
            for t, (xsT_ap, Win, Bin) in enumerate(
                [(t_xasT, "Wina", "Bina"), (t_xbsT, "Winb", "Binb")]
            ):
                for j in range(NGRP):
                    x0r, x0T = input_proj(xsT_ap, Win, Bin, t, j, src_pool, ps_pool, out_pool)
                    nc.sync.dma_start(out=t_x0s[t].ap()[j * P:(j + 1) * P, :], in_=x0r[:])
                    psq = ps_pool.tile([P, C], FP, space="PSUM", tag="psq")
                    nc.tensor.matmul(out=psq[:], lhsT=x0T[:], rhs=w_sb[f"Wq0{t}"][:],
                                     start=True, stop=True)
                    nc.vector.tensor_tensor(out=q_sb[t][:, j, :], in0=psq[:],
                                            in1=w_sb[f"Bq0{t}"][:], op=AL.add)

        # ---------- per-layer processing ----------
        def attention(r, l):
            """relation r: src type = r, dst type = 1 - r. Fills acc_sb."""
            nlo, nhi = rel_meta[r]
            lo_chunks, lo_loc = _chunks(nlo)
            hi_chunks, hi_loc = _chunks(nhi)
            lo_base = 0
            hi_base = sum(nlo)
            kv_ap = t_kv[r].ap()
            qt = q_sb[1 - r]
            drt = dr_sb[r]
            idxt = idx_sb[r]
            with (
                tc.tile_pool(name=f"gat{r}{l}", bufs=2) as gpool,
                tc.tile_pool(name=f"aps{r}{l}", bufs=2, space="PSUM") as aps,
                tc.tile_pool(name=f"accp{r}{l}", bufs=2, space="PSUM") as accp,
                tc.tile_pool(name=f"asb{r}{l}", bufs=3) as asb,
            ):
                tiles = {}

                def get_chunk(region, ci):
                    key = (region, ci)
                    if key in tiles:
                        return tiles[key]
                    chunks = lo_chunks if region == 0 else hi_chunks
                    base = lo_base if region == 0 else hi_base
                    b0, n = chunks[ci]
                    gt = gpool.tile([P, CHUNK_BLK, 2 * C], FP, tag="kvchunk")
                    in_ap = kv_ap[0:LO_LIM, :] if region == 0 else kv_ap[LO_LIM:NPAD, :]
                    if os.environ.get("SKIP_GATHER"):
                        nc.vector.memset(gt[:, 0:n, :], 1.0)
                    else:
                        nc.gpsimd.dma_gather(
                            out_ap=gt[:, 0:n, :], in_ap=in_ap,
                            idxs_ap=idxt[:, (base + b0) * 8:(base + b0 + n) * 8],
                            num_idxs=n * P, num_idxs_reg=n * P, elem_size=2 * C,
                            single_packet=False,
                        )
                    tiles[key] = gt
                    return gt

                for g in range(NGRP):
                    blks = []
                    for i, (ci, lb) in enumerate(lo_loc[g]):
                        gb = lo_base + sum(nlo[:g]) + i
                        blks.append((0, ci, lb, gb))
                    for i, (ci, lb) in enumerate(hi_loc[g]):
                        gb = hi_base + sum(nhi[:g]) + i
                        blks.append((1, ci, lb, gb))
                    accps = accp.tile([P, 132], FP, space="PSUM", tag="acc")
                    for bi, (region, ci, lb, gb) in enumerate(blks):
                        gt = get_chunk(region, ci)
                        dcol = drt[:, gb:gb + 1]
                        oh = asb.tile([P, P], FP, tag="oh")
                        nc.vector.tensor_scalar(out=oh[:], in0=iota_row[:], scalar1=dcol,
                                                scalar2=None, op0=AL.is_equal)
                        pst = aps.tile([P, P], FP, space="PSUM", tag="pst")
                        nc.tensor.transpose(out=pst[:], in_=dcol.to_broadcast([P, P]),
                                            identity=ident[:])
                        ohT = asb.tile([P, P], FP, tag="ohT")
                        nc.vector.tensor_scalar(out=ohT[:], in0=pst[:], scalar1=iota_col[:],
                                                scalar2=None, op0=AL.is_equal)
                        qg = aps.tile([P, P], FP, space="PSUM", tag="qg")
                        nc.tensor.matmul(out=qg[:], lhsT=ohT[:], rhs=qt[:, g, :],
                                         start=True, stop=True)
                        lp = asb.tile([P, P], FP, tag="lp")
                        nc.vector.tensor_tensor(out=lp[:], in0=qg[:], in1=gt[:, lb, 0:C],
                                                op=AL.mult)
                        z = asb.tile([P, H], FP, tag="z")
                        nc.vector.tensor_reduce(out=z[:], in_=lp[:].rearrange(
                            "p (h d) -> p h d", h=H), axis=mybir.AxisListType.X, op=AL.add)
                        ze = asb.tile([P, H], FP, tag="ze")
                        nc.scalar.activation(out=ze[:], in_=z[:], func=AF.Exp)
                        wz = asb.tile([P, 132], FP, tag="wz")
                        nc.vector.tensor_tensor(
                            out=wz[:, 0:C], in0=gt[:, lb, C:2 * C],
                            in1=ze[:].rearrange("p (h o) -> p h o", o=1).to_broadcast([P, H, D]),
                            op=AL.mult)
                        nc.vector.tensor_copy(wz[:, C:132], ze[:])
                        nc.tensor.matmul(out=accps[:], lhsT=oh[:], rhs=wz[:],
                                         start=(bi == 0), stop=(bi == len(blks) - 1))
                    nc.vector.tensor_copy(acc_sb[:, g, :], accps[:])

        def alin(t, l):
            """a_lin + skip for dst type t of layer l; reads acc_sb."""
            xprev = t_x0s[t] if l == 0 else t_x1s[t]
            with (
                tc.tile_pool(name=f"al{t}{l}", bufs=3) as sp,
                tc.tile_pool(name=f"alp{t}{l}", bufs=2, space="PSUM") as pp,
            ):
                for j in range(NGRP):
                    den = sp.tile([P, H], FP, tag="den")
                    nc.vector.tensor_scalar(out=den[:], in0=acc_sb[:, j, C:132],
                                            scalar1=1e-16, scalar2=None, op0=AL.add)
                    rec = sp.tile([P, H], FP, tag="rec")
                    nc.vector.reciprocal(rec[:], den[:])
                    at = sp.tile([P, C], FP, tag="at")
                    nc.vector.tensor_tensor(
                        out=at[:], in0=acc_sb[:, j, 0:C],
                        in1=rec[:].rearrange("p (h o) -> p h o", o=1).to_broadcast([P, H, D]),
                        op=AL.mult)
                    gl = sp.tile([P, C], FP, tag="gl")
                    nc.scalar.activation(out=gl[:], in_=at[:], func=AF.Gelu)
                    pst = pp.tile([P, P], FP, space="PSUM", tag="apst")
                    nc.tensor.transpose(out=pst[:], in_=gl[:], identity=ident[:])
                    glT = sp.tile([P, P], FP, tag="glT")
                    nc.vector.tensor_copy(glT[:], pst[:])
                    pso = pp.tile([P, C], FP, space="PSUM", tag="pso")
                    nc.tensor.matmul(out=pso[:], lhsT=glT[:], rhs=w_sb[f"Wal{l}{t}"][:],
                                     start=True, stop=True)
                    o1 = sp.tile([P, C], FP, tag="o1")
                    nc.vector.tensor_tensor(out=o1[:], in0=pso[:], in1=w_sb[f"Bal{l}{t}"][:],
                                            op=AL.add)
                    xp = sp.tile([P, C], FP, tag="xp")
                    nc.sync.dma_start(out=xp[:], in_=xprev.ap()[j * P:(j + 1) * P, :])
                    o2 = sp.tile([P, C], FP, tag="o2")
                    nc.vector.tensor_scalar(out=o2[:], in0=xp[:], scalar1=f[f"oms{l}{t}"],
                                            scalar2=None, op0=AL.mult)
                    nw = sp.tile([P, C], FP, tag="nw")
                    nc.vector.tensor_tensor(out=nw[:], in0=o1[:], in1=o2[:], op=AL.add)
                    if l == 0:
                        nc.sync.dma_start(out=t_x1s[t].ap()[j * P:(j + 1) * P, :], in_=nw[:])
                        pst2 = pp.tile([P, P], FP, space="PSUM", tag="apst2")
                        nc.tensor.transpose(out=pst2[:], in_=nw[:], identity=ident[:])
                        nwT = sp.tile([P, P], FP, tag="nwT")
                        nc.vector.tensor_copy(nwT[:], pst2[:])
                        nc.sync.dma_start(out=t_nxT[t].ap()[:, j * P:(j + 1) * P], in_=nwT[:])
                        psq = pp.tile([P, C], FP, space="PSUM", tag="apsq")
                        nc.tensor.matmul(out=psq[:], lhsT=nwT[:], rhs=w_sb[f"Wq1{t}"][:],
                                         start=True, stop=True)
                        nc.vector.tensor_tensor(out=q_sb[t][:, j, :], in0=psq[:],
                                                in1=w_sb[f"Bq1{t}"][:], op=AL.add)
                    else:
                        nc.sync.dma_start(out=t_out[t][j * P:(j + 1) * P, :], in_=nw[:])
                if l == 0:
                    if os.environ.get("SKIP_AG"):
                        for k in range(NCORES):
                            nc.sync.dma_start(out=t_ag[t].ap()[k, :, :], in_=t_nxT[t].ap()[:])
                    else:
                        nc.gpsimd.collective_compute(
                            "AllGather", mybir.AluOpType.bypass,
                            replica_groups=[list(range(NCORES))],
                            ins=[t_nxT[t].ap()[:]], outs=[t_ag[t].ap()[:]],
                        )

        # layer 0 attention + alin (+ AllGather inside alin)
        attention(0, 0)
        alin(1, 0)
        attention(1, 0)
        alin(0, 0)

        # layer-1 kv tables from AllGather output
        with (
            tc.tile_pool(name="p1src", bufs=3) as src_pool,
            tc.tile_pool(name="p1ps", bufs=2, space="PSUM") as ps_pool,
            tc.tile_pool(name="p1out", bufs=3) as out_pool,
        ):
            for t in range(2):
                for k in range(NCORES):
                    for j in range(NGRP):
                        lhs = src_pool.tile([P, P], FP, tag="lhs1")
                        nc.sync.dma_start(out=lhs[:], in_=t_ag[t].ap()[k, :, j * P:(j + 1) * P])
                        ps2 = ps_pool.tile([P, 2 * C], FP, space="PSUM", tag="ps2")
                        nc.tensor.matmul(out=ps2[:], lhsT=lhs[:], rhs=w_sb[f"Wkv1{t}"][:],
                                         start=True, stop=True)
                        kvt = out_pool.tile([P, 2 * C], FP, tag="kvt")
                        nc.vector.tensor_tensor(out=kvt[:], in0=ps2[:],
                                                in1=w_sb[f"Bkv1{t}"][:], op=AL.add)
                        row = k * SHARD + j * P
                        nc.sync.dma_start(out=t_kv[t].ap()[row:row + P, :], in_=kvt[:])

        attention(0, 1)
        alin(1, 1)
        attention(1, 1)
        alin(0, 1)
        cpool_cm.__exit__(None, None, None)

    nc.compile()

    in_maps = []
    for c in range(NCORES):
        m = {"xaT": xaT, "xbT": xbT,
             "xasT": np.ascontiguousarray(xaT[:, c * SHARD:(c + 1) * SHARD]),
             "xbsT": np.ascontiguousarray(xbT[:, c * SHARD:(c + 1) * SHARD]),
             "idx0": idx0[c], "dr0": dr0[c], "idx1": idx1[c], "dr1": dr1[c]}
        for n in wnames:
            m[n] = np.ascontiguousarray(f[n])
        in_maps.append(m)

    res = run_bass_kernel_spmd(
        nc, in_maps, core_ids=list(range(NCORES)),
        trace=bool(os.environ.get("BASS_TRACE")),
    )
    LAST_RESULT = res
    outa = np.concatenate([res.results[c]["out0"] for c in range(NCORES)])[:40000]
    outb = np.concatenate([res.results[c]["out1"] for c in range(NCORES)])[:40000]
    return outa, outb



# revision 5
# speedup vs baseline: 2.2135x; 2.2135x over previous
"""HGT (2-type, 2-relation, 2-layer) Bass kernel for 8 Trainium2 cores — v2.

Sharding: destination-node sharding; core c owns dst rows [5120c, 5120(c+1))
of both node types. bf16 on-chip pipeline with fp32 PSUM accumulation.

Key structure vs v1:
- Own-shard K/V projection only; full K/V tables assembled via AllGather of
  bf16 shards (per source-type per layer). K bias dropped (cancels in the
  per-dst softmax); V bias applied after normalization, before gelu.
- Per-edge gather of combined K|V rows (512B bf16) with int16 indices,
  lo/hi split at 32768.
- One-hot (oh: [edge,dst], ohT: [dst,edge]) blocks precomputed on host,
  streamed from DRAM as bf16; per-128-edge-block matmuls do the q gather
  (lhsT=ohT) and the segment-sum scatter (lhsT=oh) with fp32 PSUM accum.
- Vector work batched 4 blocks per instruction; exp/copies on the scalar
  (ACT) engine; input-proj bias folded into the matmul via a ones row.
"""
import math
import os
import sys

import numpy as np

sys.path.insert(0, "/opt/trn_rl_repo")

import ml_dtypes

BF16 = ml_dtypes.bfloat16

H, D, C, L = 4, 32, 128, 2
INV_SQRT_D = 1.0 / math.sqrt(D)
P = 128
NCORES = 8
SHARD = 5120
NGRP = SHARD // P     # 40
NPAD = NCORES * SHARD # 40960
LO_LIM = 32768
HI_ROWS = NPAD - LO_LIM
CHUNK_BLK = 16        # gather chunk size in 128-edge blocks (multiple of 4)
B = 4                 # vector batch size in blocks

LAST_RESULT = None


def _ceil4(x):
    return (x + 3) // 4 * 4


def _fold_weights(ins):
    """Fold a_rel/m_rel into k/v weights, p_rel/sqrt(D) into q, sigmoid(skip)
    into a_lin. K bias dropped (softmax-invariant); V bias kept separately
    (applied post-normalization). Returns dict of numpy arrays (bf16)."""
    f = {}
    for l in range(L):
        for t in range(2):
            kw = np.asarray(ins["k_w"][l, t], np.float32)   # [C, C]
            kb = np.asarray(ins["k_b"][l, t], np.float32)
            vw = np.asarray(ins["v_w"][l, t], np.float32)
            vb = np.asarray(ins["v_b"][l, t], np.float32)
            ar = np.asarray(ins["a_rel"][l, t], np.float32)  # [H, D, D]
            mr = np.asarray(ins["m_rel"][l, t], np.float32)
            wk = np.zeros((C, C), np.float32)
            wv = np.zeros((C, C), np.float32)
            bv = np.zeros(C, np.float32)
            for h in range(H):
                sl = slice(h * D, (h + 1) * D)
                wk[:, sl] = kw[:, sl] @ ar[h]
                wv[:, sl] = vw[:, sl] @ mr[h]
                bv[sl] = vb[sl] @ mr[h]
            del kb
            f[f"Wkv{l}{t}"] = np.concatenate([wk, wv], axis=1).astype(BF16)  # [C,2C]
            # relation t's dst type is 1-t: bv applied in alin(1-t, l)
            f[f"Bv{l}{1 - t}"] = np.tile(bv[None, :], (P, 1)).astype(BF16)
            r_dst = 1 - t
            pr = np.asarray(ins["p_rel"][l, r_dst], np.float32) * INV_SQRT_D
            scale = np.repeat(pr, D)
            f[f"Wq{l}{t}"] = (np.asarray(ins["q_w"][l, t], np.float32) * scale[None, :]).astype(BF16)
            f[f"Bq{l}{t}"] = np.tile((np.asarray(ins["q_b"][l, t], np.float32) * scale)[None, :], (P, 1)).astype(BF16)
            s = 1.0 / (1.0 + math.exp(-float(np.asarray(ins["skip"][l, t]))))
            f[f"Wal{l}{t}"] = (np.asarray(ins["a_lin_w"][l, t], np.float32) * s).astype(BF16)
            f[f"Bal{l}{t}"] = np.tile((np.asarray(ins["a_lin_b"][l, t], np.float32) * s)[None, :], (P, 1)).astype(BF16)
            f[f"oms{l}{t}"] = 1.0 - s
    # input linears with folded bias row (ones appended to lhsT on host)
    wina = np.asarray(ins["lin_a_w"], np.float32)   # [64, C]
    bina = np.asarray(ins["lin_a_b"], np.float32)
    winb = np.asarray(ins["lin_b_w"], np.float32)   # [32, C]
    binb = np.asarray(ins["lin_b_b"], np.float32)
    f["Wina"] = np.concatenate([wina, bina[None, :]], 0).astype(BF16)  # [65, C]
    f["Winb"] = np.concatenate([winb, binb[None, :]], 0).astype(BF16)  # [33, C]
    return f


def _prep_edges(edge):
    """Partition one relation's edges by dst shard. Returns
    (idx_w[8], oh[8], ohT[8], sched) where sched describes the shared static
    block schedule: dict with nlo, nhi, TLp, THp, TB, and per-block
    (group, first, last) info per region."""
    src = np.asarray(edge[0]).astype(np.int64)
    dst = np.asarray(edge[1]).astype(np.int64)
    core = dst // SHARD
    nlo = np.zeros(NGRP, np.int64)
    nhi = np.zeros(NGRP, np.int64)
    percore = []
    for c in range(NCORES):
        m = core == c
        s, dl = src[m], dst[m] - c * SHARD
        g = dl // P
        rel = dl % P
        lo = s < LO_LIM
        percore.append((s, g, rel, lo))
        for gi in range(NGRP):
            gm = g == gi
            nlo[gi] = max(nlo[gi], int(np.sum(gm & lo)))
            nhi[gi] = max(nhi[gi], int(np.sum(gm & ~lo)))
    nlo = np.maximum((nlo + P - 1) // P, 1)           # blocks per group, >=1
    nhi = (nhi + P - 1) // P
    TL, TH = int(nlo.sum()), int(nhi.sum())
    TLp, THp = _ceil4(TL), _ceil4(TH)
    TB = TLp + THp
    lo_off = np.concatenate([[0], np.cumsum(nlo)[:-1]])
    hi_off = np.concatenate([[0], np.cumsum(nhi)[:-1]]) + TLp

    # per-block group assignment (pads attach to last group)
    blk_grp = np.zeros(TB, np.int64)
    for gi in range(NGRP):
        blk_grp[lo_off[gi]:lo_off[gi] + nlo[gi]] = gi
        blk_grp[hi_off[gi]:hi_off[gi] + nhi[gi]] = gi
    blk_grp[TL:TLp] = NGRP - 1
    blk_grp[TLp + TH:TB] = NGRP - 1

    idx_ws, ohs, ohTs = [], [], []
    for c in range(NCORES):
        s, g, rel, lo = percore[c]
        idx = np.zeros(TB * P, np.int16)
        dr = np.full(TB * P, -1.0, np.float32)
        for gi in range(NGRP):
            for reg, off in ((True, lo_off[gi]), (False, hi_off[gi])):
                gm = (g == gi) & (lo == reg)
                sg, rg = s[gm], rel[gm]
                o = np.argsort(sg, kind="stable")
                sg, rg = sg[o], rg[o]
                base = int(off) * P
                idx[base:base + len(sg)] = (sg if reg else sg - LO_LIM).astype(np.int16)
                dr[base:base + len(sg)] = rg
        idx_ws.append(np.tile(idx.reshape(TB * P // 16, 16).T, (8, 1)).copy())
        drb = dr.reshape(TB, P)                                   # [blk, e]
        j = np.arange(P, dtype=np.float32)
        oh = (drb[:, :, None] == j[None, None, :])                # [blk, e, j]
        ohs.append(np.ascontiguousarray(
            oh.transpose(1, 0, 2).reshape(P, TB * P)).astype(BF16))
        ohT = (drb[:, None, :] == j[None, :, None])               # [blk, j, e]
        ohTs.append(np.ascontiguousarray(
            ohT.transpose(1, 0, 2).reshape(P, TB * P)).astype(BF16))
    sched = dict(nlo=nlo.tolist(), nhi=nhi.tolist(),
                 lo_off=lo_off.tolist(), hi_off=hi_off.tolist(),
                 TL=TL, TH=TH, TLp=TLp, THp=THp, TB=TB,
                 blk_grp=blk_grp.tolist())
    return idx_ws, ohs, ohTs, sched


def kernel(**ins):
    global LAST_RESULT
    import concourse.bass as bass
    import concourse.tile as tile
    from concourse import bacc, mybir
    from concourse.bass_utils import run_bass_kernel_spmd

    FP = mybir.dt.float32
    BF = mybir.dt.bfloat16
    I16 = mybir.dt.int16
    AL = mybir.AluOpType
    AF = mybir.ActivationFunctionType

    f = _fold_weights(ins)
    idx0, oh0, ohT0, sc0 = _prep_edges(np.asarray(ins["edge_ab"]))
    idx1, oh1, ohT1, sc1 = _prep_edges(np.asarray(ins["edge_ba"]))
    scheds = [sc0, sc1]
    TBs = [sc0["TB"], sc1["TB"]]

    xa = np.asarray(ins["x_a"], np.float32)
    xb = np.asarray(ins["x_b"], np.float32)
    DA, DB = xa.shape[1], xb.shape[1]
    # transposed, zero-padded, ones row appended (bias fold), bf16
    xaT = np.zeros((DA + 1, NPAD), np.float32)
    xaT[:DA, :40000] = xa.T
    xaT[DA, :] = 1.0
    xbT = np.zeros((DB + 1, NPAD), np.float32)
    xbT[:DB, :40000] = xb.T
    xbT[DB, :] = 1.0
    xaT = xaT.astype(BF16)
    xbT = xbT.astype(BF16)

    nc = bacc.Bacc("TRN2", target_bir_lowering=False, debug=False, num_devices=NCORES)

    # ---- DRAM tensors ----
    t_xasT = nc.dram_tensor("xasT", [DA + 1, SHARD], BF, kind="ExternalInput").ap()
    t_xbsT = nc.dram_tensor("xbsT", [DB + 1, SHARD], BF, kind="ExternalInput").ap()
    wnames = ["Wina", "Winb"]
    for l in range(L):
        for t in range(2):
            wnames += [f"Wkv{l}{t}", f"Wq{l}{t}", f"Bq{l}{t}",
                       f"Wal{l}{t}", f"Bal{l}{t}", f"Bv{l}{t}"]
    t_w = {n: nc.dram_tensor(n, list(f[n].shape), BF, kind="ExternalInput").ap()
           for n in wnames}
    t_idx = [nc.dram_tensor(f"idx{r}", [P, TBs[r] * 8], I16, kind="ExternalInput").ap()
             for r in range(2)]
    t_oh = [nc.dram_tensor(f"oh{r}", [P, TBs[r] * P], BF, kind="ExternalInput").ap()
            for r in range(2)]
    t_ohT = [nc.dram_tensor(f"ohT{r}", [P, TBs[r] * P], BF, kind="ExternalInput").ap()
             for r in range(2)]

    # K|V tables per (src type, layer): AllGather output, viewed flat for gathers
    t_tab = [[nc.dram_tensor(f"tab{t}{l}", [NCORES, SHARD, 2 * C], BF,
                             addr_space="Shared") for l in range(L)]
             for t in range(2)]
    t_agsrc = [[nc.dram_tensor(f"agsrc{t}{l}", [SHARD, 2 * C], BF)
                for l in range(L)] for t in range(2)]
    t_out = [nc.dram_tensor(f"out{t}", [SHARD, C], FP, kind="ExternalOutput").ap()
             for t in range(2)]

    with tile.TileContext(nc) as tc:
        cpool_cm = tc.tile_pool(name="const", bufs=1)
        cpool = cpool_cm.__enter__()
        ident = cpool.tile([P, P], BF)
        from concourse.masks import make_identity
        make_identity(nc, ident[:])
        w_sb = {}
        for n in wnames:
            w_sb[n] = cpool.tile(list(f[n].shape), BF, name=n, tag=n)
            nc.sync.dma_start(out=w_sb[n][:], in_=t_w[n][:])
        idx_sb = []
        for r in range(2):
            it = cpool.tile([P, TBs[r] * 8], I16, name=f"idxsb{r}", tag=f"idxsb{r}")
            nc.sync.dma_start(out=it[:], in_=t_idx[r][:])
            idx_sb.append(it)
        # persistent per-shard state
        q_sb = [cpool.tile([P, NGRP, C], BF, name=f"qsb{t}", tag=f"qsb{t}")
                for t in range(2)]
        x_sb = [[cpool.tile([P, NGRP, C], BF, name=f"xsb{t}{l}", tag=f"xsb{t}{l}")
                 for l in range(2)] for t in range(2)]
        acc_sb = [cpool.tile([P, NGRP, 132], FP, name=f"accsb{t}", tag=f"accsb{t}")
                  for t in range(2)]
        xsT_sb = {}
        for t, (ap_, din) in enumerate([(t_xasT, DA + 1), (t_xbsT, DB + 1)]):
            xt = cpool.tile([din, SHARD], BF, name=f"xsT{t}", tag=f"xsT{t}")
            nc.sync.dma_start(out=xt[:], in_=ap_[:])
            xsT_sb[t] = xt

        def ag(t, l):
            if os.environ.get("SKIP_AG"):
                for k in range(NCORES):
                    nc.sync.dma_start(out=t_tab[t][l].ap()[k, :, :],
                                      in_=t_agsrc[t][l].ap()[:])
            else:
                nc.gpsimd.collective_compute(
                    "AllGather", mybir.AluOpType.bypass,
                    replica_groups=[list(range(NCORES))],
                    ins=[t_agsrc[t][l].ap()[:]], outs=[t_tab[t][l].ap()[:]],
                )

        # ---------- phase 1: layer-0 own-shard projections ----------
        def phase1(t):
            Win = "Wina" if t == 0 else "Winb"
            din = (DA if t == 0 else DB) + 1
            with (
                tc.tile_pool(name=f"p1s{t}", bufs=3) as sp,
                tc.tile_pool(name=f"p1p{t}", bufs=2, space="PSUM") as pp,
            ):
                for g in range(NGRP):
                    ps0 = pp.tile([P, C], FP, space="PSUM", tag="c1")
                    nc.tensor.matmul(out=ps0[:], lhsT=xsT_sb[t][:, g * P:(g + 1) * P],
                                     rhs=w_sb[Win][:], start=True, stop=True)
                    # relu + cast into resident x0
                    nc.scalar.activation(out=x_sb[t][0][:, g, :], in_=ps0[:], func=AF.Relu)
                    pst = pp.tile([P, P], BF, space="PSUM", tag="pst")
                    nc.tensor.transpose(out=pst[:], in_=x_sb[t][0][:, g, :], identity=ident[:])
                    x0T = sp.tile([P, P], BF, tag="x0T")
                    nc.scalar.activation(out=x0T[:], in_=pst[:], func=AF.Copy)
                    pkv = pp.tile([P, 2 * C], FP, space="PSUM", tag="c2")
                    nc.tensor.matmul(out=pkv[:], lhsT=x0T[:], rhs=w_sb[f"Wkv0{t}"][:],
                                     start=True, stop=True)
                    kvt = sp.tile([P, 2 * C], BF, tag="kvt")
                    nc.vector.tensor_copy(kvt[:], pkv[:])
                    nc.sync.dma_start(out=t_agsrc[t][0].ap()[g * P:(g + 1) * P, :], in_=kvt[:])
                    pq = pp.tile([P, C], FP, space="PSUM", tag="c1")
                    nc.tensor.matmul(out=pq[:], lhsT=x0T[:], rhs=w_sb[f"Wq0{t}"][:],
                                     start=True, stop=True)
                    nc.vector.tensor_tensor(out=q_sb[t][:, g, :], in0=pq[:],
                                            in1=w_sb[f"Bq0{t}"][:], op=AL.add)

        # ---------- attention ----------
        def attention(r, l):
            """relation r: src type r, dst type 1-r; fills acc_sb[1-r]."""
            sc = scheds[r]
            td = 1 - r
            tabflat = t_tab[r][l].ap().rearrange("k n c -> (k n) c")
            qt = q_sb[td]
            idxt = idx_sb[r]
            blk_grp = sc["blk_grp"]
            with (
                tc.tile_pool(name=f"gat{r}{l}", bufs=2) as gpool,
                tc.tile_pool(name=f"bat{r}{l}", bufs=3) as bpool,
                tc.tile_pool(name=f"aps{r}{l}", bufs=2, space="PSUM") as aps,
                tc.tile_pool(name=f"accp{r}{l}", bufs=2, space="PSUM") as accp,
            ):
                for region in range(2):
                    r0 = 0 if region == 0 else sc["TLp"]
                    r1 = sc["TLp"] if region == 0 else sc["TB"]
                    nblk_reg = r1 - r0
                    if nblk_reg == 0:
                        continue
                    in_ap = tabflat[0:LO_LIM, :] if region == 0 else tabflat[LO_LIM:NPAD, :]
                    accps = None
                    cur_grp = -1
                    for c0 in range(r0, r1, CHUNK_BLK):
                        n = min(CHUNK_BLK, r1 - c0)
                        gt = gpool.tile([P, CHUNK_BLK, 2 * C], BF, tag="kvchunk")
                        if os.environ.get("SKIP_GATHER"):
                            nc.vector.memset(gt[:, 0:n, :], 1.0)
                        else:
                            nc.gpsimd.dma_gather(
                                out_ap=gt[:, 0:n, :], in_ap=in_ap,
                                idxs_ap=idxt[:, c0 * 8:(c0 + n) * 8],
                                num_idxs=n * P, num_idxs_reg=n * P,
                                elem_size=2 * C, single_packet=False,
                            )
                        oht_c = gpool.tile([P, CHUNK_BLK, P], BF, tag="ohTchunk")
                        nc.sync.dma_start(
                            out=oht_c[:, 0:n, :].rearrange("p a b -> p (a b)"),
                            in_=t_ohT[r][:, c0 * P:(c0 + n) * P])
                        oh_c = gpool.tile([P, CHUNK_BLK, P], BF, tag="ohchunk")
                        nc.sync.dma_start(
                            out=oh_c[:, 0:n, :].rearrange("p a b -> p (a b)"),
                            in_=t_oh[r][:, c0 * P:(c0 + n) * P])
                        for b0 in range(0, n, B):
                            nb = min(B, n - b0)
                            qg_ps = aps.tile([P, B, C], FP, space="PSUM", tag="qg")
                            for i in range(nb):
                                g = blk_grp[c0 + b0 + i]
                                nc.tensor.matmul(out=qg_ps[:, i, :],
                                                 lhsT=oht_c[:, b0 + i, :],
                                                 rhs=qt[:, g, :], start=True, stop=True)
                            qg = bpool.tile([P, B, C], BF, tag="qg_sb")
                            nc.scalar.activation(out=qg[:, 0:nb, :], in_=qg_ps[:, 0:nb, :],
                                                 func=AF.Copy)
                            lp = bpool.tile([P, B, C], BF, tag="lp")
                            nc.vector.tensor_tensor(out=lp[:, 0:nb, :], in0=qg[:, 0:nb, :],
                                                    in1=gt[:, b0:b0 + nb, 0:C], op=AL.mult)
                            z = bpool.tile([P, B * H], FP, tag="z")
                            nc.vector.tensor_reduce(
                                out=z[:, 0:nb * H],
                                in_=lp[:, 0:nb, :].rearrange("p b (h d) -> p (b h) d", h=H),
                                axis=mybir.AxisListType.X, op=AL.add)
                            ze = bpool.tile([P, B * H], BF, tag="ze")
                            nc.scalar.activation(out=ze[:, 0:nb * H], in_=z[:, 0:nb * H],
                                                 func=AF.Exp)
                            zx = bpool.tile([P, B, C], BF, tag="zx")
                            nc.scalar.activation(
                                out=zx[:, 0:nb, :].rearrange("p b (h d) -> p b h d", h=H),
                                in_=ze[:, 0:nb * H].rearrange("p (b h) -> p b h ()", h=H)
                                    .to_broadcast([P, nb, H, D]),
                                func=AF.Copy)
                            wz = bpool.tile([P, B, 132], BF, tag="wz")
                            nc.vector.tensor_tensor(out=wz[:, 0:nb, 0:C],
                                                    in0=gt[:, b0:b0 + nb, C:2 * C],
                                                    in1=zx[:, 0:nb, :], op=AL.mult)
                            nc.vector.tensor_copy(
                                wz[:, 0:nb, C:C + H],
                                ze[:, 0:nb * H].rearrange("p (b h) -> p b h", h=H))
                            for i in range(nb):
                                blk = c0 + b0 + i
                                g = blk_grp[blk]
                                if g != cur_grp:
                                    accps = accp.tile([P, 132], FP, space="PSUM", tag="acc")
                                    cur_grp = g
                                off = sc["lo_off"][g] if region == 0 else sc["hi_off"][g]
                                cnt = sc["nlo"][g] if region == 0 else sc["nhi"][g]
                                end = off + cnt
                                if g == NGRP - 1:
                                    end = r1    # pads attach to last group
                                first = blk == off
                                last = blk == end - 1
                                nc.tensor.matmul(out=accps[:], lhsT=oh_c[:, b0 + i, :],
                                                 rhs=wz[:, i, :], start=first, stop=last)
                                if last:
                                    if region == 0:
                                        nc.vector.tensor_copy(acc_sb[td][:, g, :], accps[:])
                                    else:
                                        nc.vector.tensor_tensor(
                                            out=acc_sb[td][:, g, :], in0=accps[:],
                                            in1=acc_sb[td][:, g, :], op=AL.add)

        # ---------- alin ----------
        def alin(t, l):
            """a_lin + skip for dst type t, layer l; reads acc_sb[t]. For l=0
            also produces layer-1 q, resident x1, and the layer-1 K|V shard."""
            with (
                tc.tile_pool(name=f"al{t}{l}", bufs=3) as sp,
                tc.tile_pool(name=f"alp{t}{l}", bufs=1, space="PSUM") as pp,
            ):
                for g in range(NGRP):
                    den = sp.tile([P, H], FP, tag="den")
                    nc.vector.tensor_scalar(out=den[:], in0=acc_sb[t][:, g, C:C + H],
                                            scalar1=1e-16, scalar2=None, op0=AL.add)
                    rec = sp.tile([P, H], FP, tag="rec")
                    nc.vector.reciprocal(rec[:], den[:])
                    at = sp.tile([P, C], BF, tag="at")
                    nc.vector.tensor_tensor(
                        out=at[:], in0=acc_sb[t][:, g, 0:C],
                        in1=rec[:].rearrange("p (h o) -> p h o", o=1).to_broadcast([P, H, D]),
                        op=AL.mult)
                    at2 = sp.tile([P, C], BF, tag="at2")
                    nc.vector.tensor_tensor(out=at2[:], in0=at[:], in1=w_sb[f"Bv{l}{t}"][:],
                                            op=AL.add)
                    gl = sp.tile([P, C], BF, tag="gl")
                    nc.scalar.activation(out=gl[:], in_=at2[:], func=AF.Gelu)
                    pst = pp.tile([P, P], BF, space="PSUM", tag="trans")
                    nc.tensor.transpose(out=pst[:], in_=gl[:], identity=ident[:])
                    glT = sp.tile([P, P], BF, tag="glT")
                    nc.scalar.activation(out=glT[:], in_=pst[:], func=AF.Copy)
                    pso = pp.tile([P, C], FP, space="PSUM", tag="c1")
                    nc.tensor.matmul(out=pso[:], lhsT=glT[:], rhs=w_sb[f"Wal{l}{t}"][:],
                                     start=True, stop=True)
                    o1 = sp.tile([P, C], BF, tag="o1")
                    nc.vector.tensor_tensor(out=o1[:], in0=pso[:], in1=w_sb[f"Bal{l}{t}"][:],
                                            op=AL.add)
                    o2 = sp.tile([P, C], BF, tag="o2")
                    nc.vector.tensor_scalar(out=o2[:], in0=x_sb[t][l][:, g, :],
                                            scalar1=f[f"oms{l}{t}"], scalar2=None,
                                            op0=AL.mult)
                    if l == 0:
                        nw = x_sb[t][1][:, g, :]
                        nc.vector.tensor_tensor(out=nw, in0=o1[:], in1=o2[:], op=AL.add)
                        pst2 = pp.tile([P, P], BF, space="PSUM", tag="trans")
                        nc.tensor.transpose(out=pst2[:], in_=nw, identity=ident[:])
                        nwT = sp.tile([P, P], BF, tag="nwT")
                        nc.scalar.activation(out=nwT[:], in_=pst2[:], func=AF.Copy)
                        pkv = pp.tile([P, 2 * C], FP, space="PSUM", tag="c2")
                        nc.tensor.matmul(out=pkv[:], lhsT=nwT[:], rhs=w_sb[f"Wkv1{t}"][:],
                                         start=True, stop=True)
                        kvt = sp.tile([P, 2 * C], BF, tag="alkvt")
                        nc.vector.tensor_copy(kvt[:], pkv[:])
                        nc.sync.dma_start(out=t_agsrc[t][1].ap()[g * P:(g + 1) * P, :],
                                          in_=kvt[:])
                        pq = pp.tile([P, C], FP, space="PSUM", tag="c1")
                        nc.tensor.matmul(out=pq[:], lhsT=nwT[:], rhs=w_sb[f"Wq1{t}"][:],
                                         start=True, stop=True)
                        nc.vector.tensor_tensor(out=q_sb[t][:, g, :], in0=pq[:],
                                                in1=w_sb[f"Bq1{t}"][:], op=AL.add)
                    else:
                        nw = sp.tile([P, C], FP, tag="nwf")
                        nc.vector.tensor_tensor(out=nw[:], in0=o1[:], in1=o2[:], op=AL.add)
                        nc.sync.dma_start(out=t_out[t][g * P:(g + 1) * P, :], in_=nw[:])

        # ---------- schedule ----------
        phase1(0)
        ag(0, 0)
        phase1(1)
        attention(0, 0)     # dst b, table a/l0
        ag(1, 0)
        alin(1, 0)          # x1_b, kv_b_l1 shard
        attention(1, 0)     # dst a, table b/l0
        ag(1, 1)
        alin(0, 0)          # x1_a, kv_a_l1 shard
        attention(1, 1)     # dst a, table b/l1
        ag(0, 1)
        alin(0, 1)          # out a
        attention(0, 1)     # dst b, table a/l1
        alin(1, 1)          # out b
        cpool_cm.__exit__(None, None, None)

    nc.compile()

    in_maps = []
    for c in range(NCORES):
        m = {"xasT": np.ascontiguousarray(xaT[:, c * SHARD:(c + 1) * SHARD]),
             "xbsT": np.ascontiguousarray(xbT[:, c * SHARD:(c + 1) * SHARD]),
             "idx0": idx0[c], "oh0": oh0[c], "ohT0": ohT0[c],
             "idx1": idx1[c], "oh1": oh1[c], "ohT1": ohT1[c]}
        for n in wnames:
            m[n] = np.ascontiguousarray(f[n])
        in_maps.append(m)

    res = run_bass_kernel_spmd(
        nc, in_maps, core_ids=list(range(NCORES)),
        trace=bool(os.environ.get("BASS_TRACE")),
    )
    LAST_RESULT = res
    outa = np.concatenate([res.results[c]["out0"] for c in range(NCORES)])[:40000]
    outb = np.concatenate([res.results[c]["out1"] for c in range(NCORES)])[:40000]
    return outa, outb


# revision 7
# speedup vs baseline: 2.2849x; 1.0323x over previous
"""HGT (2-type, 2-relation, 2-layer) Bass kernel for 8 Trainium2 cores — v2.

Sharding: destination-node sharding; core c owns dst rows [5120c, 5120(c+1))
of both node types. bf16 on-chip pipeline with fp32 PSUM accumulation.

Key structure vs v1:
- Own-shard K/V projection only; full K/V tables assembled via AllGather of
  bf16 shards (per source-type per layer). K bias dropped (cancels in the
  per-dst softmax); V bias applied after normalization, before gelu.
- Per-edge gather of combined K|V rows (512B bf16) with int16 indices,
  lo/hi split at 32768.
- One-hot (oh: [edge,dst], ohT: [dst,edge]) blocks precomputed on host,
  streamed from DRAM as bf16; per-128-edge-block matmuls do the q gather
  (lhsT=ohT) and the segment-sum scatter (lhsT=oh) with fp32 PSUM accum.
- Vector work batched 4 blocks per instruction; exp/copies on the scalar
  (ACT) engine; input-proj bias folded into the matmul via a ones row.
"""
import math
import os
import sys

import numpy as np

sys.path.insert(0, "/opt/trn_rl_repo")

import ml_dtypes

BF16 = ml_dtypes.bfloat16

H, D, C, L = 4, 32, 128, 2
INV_SQRT_D = 1.0 / math.sqrt(D)
P = 128
NCORES = 8
SHARD = 5120
NGRP = SHARD // P     # 40
NPAD = NCORES * SHARD # 40960
LO_LIM = 32768
HI_ROWS = NPAD - LO_LIM
CHUNK_BLK = 16        # gather chunk size in 128-edge blocks (multiple of 4)
B = 4                 # vector batch size in blocks

LAST_RESULT = None


def _ceil4(x):
    return (x + 3) // 4 * 4


def _fold_weights(ins):
    """Fold a_rel/m_rel into k/v weights, p_rel/sqrt(D) into q, sigmoid(skip)
    into a_lin. K bias dropped (softmax-invariant); V bias kept separately
    (applied post-normalization). Returns dict of numpy arrays (bf16)."""
    f = {}
    for l in range(L):
        for t in range(2):
            kw = np.asarray(ins["k_w"][l, t], np.float32)   # [C, C]
            kb = np.asarray(ins["k_b"][l, t], np.float32)
            vw = np.asarray(ins["v_w"][l, t], np.float32)
            vb = np.asarray(ins["v_b"][l, t], np.float32)
            ar = np.asarray(ins["a_rel"][l, t], np.float32)  # [H, D, D]
            mr = np.asarray(ins["m_rel"][l, t], np.float32)
            wk = np.zeros((C, C), np.float32)
            wv = np.zeros((C, C), np.float32)
            bv = np.zeros(C, np.float32)
            for h in range(H):
                sl = slice(h * D, (h + 1) * D)
                wk[:, sl] = kw[:, sl] @ ar[h]
                wv[:, sl] = vw[:, sl] @ mr[h]
                bv[sl] = vb[sl] @ mr[h]
            del kb
            f[f"Wkv{l}{t}"] = np.concatenate([wk, wv], axis=1).astype(BF16)  # [C,2C]
            # relation t's dst type is 1-t: bv applied in alin(1-t, l)
            f[f"Bv{l}{1 - t}"] = np.tile(bv[None, :], (P, 1)).astype(BF16)
            r_dst = 1 - t
            pr = np.asarray(ins["p_rel"][l, r_dst], np.float32) * INV_SQRT_D
            scale = np.repeat(pr, D)
            f[f"Wq{l}{t}"] = (np.asarray(ins["q_w"][l, t], np.float32) * scale[None, :]).astype(BF16)
            f[f"Bq{l}{t}"] = np.tile((np.asarray(ins["q_b"][l, t], np.float32) * scale)[None, :], (P, 1)).astype(BF16)
            s = 1.0 / (1.0 + math.exp(-float(np.asarray(ins["skip"][l, t]))))
            f[f"Wal{l}{t}"] = (np.asarray(ins["a_lin_w"][l, t], np.float32) * s).astype(BF16)
            f[f"Bal{l}{t}"] = np.tile((np.asarray(ins["a_lin_b"][l, t], np.float32) * s)[None, :], (P, 1)).astype(BF16)
            f[f"oms{l}{t}"] = 1.0 - s
    # input linears with folded bias row (ones appended to lhsT on host)
    wina = np.asarray(ins["lin_a_w"], np.float32)   # [64, C]
    bina = np.asarray(ins["lin_a_b"], np.float32)
    winb = np.asarray(ins["lin_b_w"], np.float32)   # [32, C]
    binb = np.asarray(ins["lin_b_b"], np.float32)
    f["Wina"] = np.concatenate([wina, bina[None, :]], 0).astype(BF16)  # [65, C]
    f["Winb"] = np.concatenate([winb, binb[None, :]], 0).astype(BF16)  # [33, C]
    return f


def _prep_edges(edge):
    """Partition one relation's edges by dst shard. Returns
    (idx_w[8], oh[8], ohT[8], sched) where sched describes the shared static
    block schedule: dict with nlo, nhi, TLp, THp, TB, and per-block
    (group, first, last) info per region."""
    src = np.asarray(edge[0]).astype(np.int64)
    dst = np.asarray(edge[1]).astype(np.int64)
    core = dst // SHARD
    nlo = np.zeros(NGRP, np.int64)
    nhi = np.zeros(NGRP, np.int64)
    percore = []
    for c in range(NCORES):
        m = core == c
        s, dl = src[m], dst[m] - c * SHARD
        g = dl // P
        rel = dl % P
        lo = s < LO_LIM
        percore.append((s, g, rel, lo))
        for gi in range(NGRP):
            gm = g == gi
            nlo[gi] = max(nlo[gi], int(np.sum(gm & lo)))
            nhi[gi] = max(nhi[gi], int(np.sum(gm & ~lo)))
    nlo = np.maximum((nlo + P - 1) // P, 1)           # blocks per group, >=1
    nhi = (nhi + P - 1) // P
    TL, TH = int(nlo.sum()), int(nhi.sum())
    TLp, THp = _ceil4(TL), _ceil4(TH)
    TB = TLp + THp
    lo_off = np.concatenate([[0], np.cumsum(nlo)[:-1]])
    hi_off = np.concatenate([[0], np.cumsum(nhi)[:-1]]) + TLp

    # per-block group assignment (pads attach to last group)
    blk_grp = np.zeros(TB, np.int64)
    for gi in range(NGRP):
        blk_grp[lo_off[gi]:lo_off[gi] + nlo[gi]] = gi
        blk_grp[hi_off[gi]:hi_off[gi] + nhi[gi]] = gi
    blk_grp[TL:TLp] = NGRP - 1
    blk_grp[TLp + TH:TB] = NGRP - 1

    idx_ws, ohs, ohTs = [], [], []
    for c in range(NCORES):
        s, g, rel, lo = percore[c]
        idx = np.zeros(TB * P, np.int16)
        dr = np.full(TB * P, -1.0, np.float32)
        for gi in range(NGRP):
            for reg, off in ((True, lo_off[gi]), (False, hi_off[gi])):
                gm = (g == gi) & (lo == reg)
                sg, rg = s[gm], rel[gm]
                o = np.argsort(sg, kind="stable")
                sg, rg = sg[o], rg[o]
                base = int(off) * P
                idx[base:base + len(sg)] = (sg if reg else sg - LO_LIM).astype(np.int16)
                dr[base:base + len(sg)] = rg
        idx_ws.append(np.tile(idx.reshape(TB * P // 16, 16).T, (8, 1)).copy())
        drb = dr.reshape(TB, P)                                   # [blk, e]
        j = np.arange(P, dtype=np.float32)
        oh = (drb[:, :, None] == j[None, None, :])                # [blk, e, j]
        ohs.append(np.ascontiguousarray(
            oh.transpose(1, 0, 2).reshape(P, TB * P)).astype(BF16))
        ohT = (drb[:, None, :] == j[None, :, None])               # [blk, j, e]
        ohTs.append(np.ascontiguousarray(
            ohT.transpose(1, 0, 2).reshape(P, TB * P)).astype(BF16))
    sched = dict(nlo=nlo.tolist(), nhi=nhi.tolist(),
                 lo_off=lo_off.tolist(), hi_off=hi_off.tolist(),
                 TL=TL, TH=TH, TLp=TLp, THp=THp, TB=TB,
                 blk_grp=blk_grp.tolist())
    return idx_ws, ohs, ohTs, sched


def kernel(**ins):
    global LAST_RESULT
    import concourse.bass as bass
    import concourse.tile as tile
    from concourse import bacc, mybir
    from concourse.bass_utils import run_bass_kernel_spmd

    FP = mybir.dt.float32
    BF = mybir.dt.bfloat16
    I16 = mybir.dt.int16
    AL = mybir.AluOpType
    AF = mybir.ActivationFunctionType

    f = _fold_weights(ins)
    idx0, oh0, ohT0, sc0 = _prep_edges(np.asarray(ins["edge_ab"]))
    idx1, oh1, ohT1, sc1 = _prep_edges(np.asarray(ins["edge_ba"]))
    scheds = [sc0, sc1]
    TBs = [sc0["TB"], sc1["TB"]]

    xa = np.asarray(ins["x_a"], np.float32)
    xb = np.asarray(ins["x_b"], np.float32)
    DA, DB = xa.shape[1], xb.shape[1]
    # transposed, zero-padded, ones row appended (bias fold), bf16
    xaT = np.zeros((DA + 1, NPAD), np.float32)
    xaT[:DA, :40000] = xa.T
    xaT[DA, :] = 1.0
    xbT = np.zeros((DB + 1, NPAD), np.float32)
    xbT[:DB, :40000] = xb.T
    xbT[DB, :] = 1.0
    xaT = xaT.astype(BF16)
    xbT = xbT.astype(BF16)

    nc = bacc.Bacc("TRN2", target_bir_lowering=False, debug=False, num_devices=NCORES)

    # ---- DRAM tensors ----
    t_xasT = nc.dram_tensor("xasT", [DA + 1, SHARD], BF, kind="ExternalInput").ap()
    t_xbsT = nc.dram_tensor("xbsT", [DB + 1, SHARD], BF, kind="ExternalInput").ap()
    wnames = ["Wina", "Winb"]
    for l in range(L):
        for t in range(2):
            wnames += [f"Wkv{l}{t}", f"Wq{l}{t}", f"Bq{l}{t}",
                       f"Wal{l}{t}", f"Bal{l}{t}", f"Bv{l}{t}"]
    t_w = {n: nc.dram_tensor(n, list(f[n].shape), BF, kind="ExternalInput").ap()
           for n in wnames}
    t_idx = [nc.dram_tensor(f"idx{r}", [P, TBs[r] * 8], I16, kind="ExternalInput").ap()
             for r in range(2)]
    t_oh = [nc.dram_tensor(f"oh{r}", [P, TBs[r] * P], BF, kind="ExternalInput").ap()
            for r in range(2)]
    t_ohT = [nc.dram_tensor(f"ohT{r}", [P, TBs[r] * P], BF, kind="ExternalInput").ap()
             for r in range(2)]

    # K|V tables per (src type, layer): AllGather output, viewed flat for gathers
    t_tab = [[nc.dram_tensor(f"tab{t}{l}", [NCORES, SHARD, 2 * C], BF,
                             addr_space="Shared") for l in range(L)]
             for t in range(2)]
    t_agsrc = [[nc.dram_tensor(f"agsrc{t}{l}", [SHARD, 2 * C], BF)
                for l in range(L)] for t in range(2)]
    t_out = [nc.dram_tensor(f"out{t}", [SHARD, C], FP, kind="ExternalOutput").ap()
             for t in range(2)]

    with tile.TileContext(nc) as tc:
        cpool_cm = tc.tile_pool(name="const", bufs=1)
        cpool = cpool_cm.__enter__()
        ident = cpool.tile([P, P], BF)
        from concourse.masks import make_identity
        make_identity(nc, ident[:])
        w_sb = {}
        for n in wnames:
            w_sb[n] = cpool.tile(list(f[n].shape), BF, name=n, tag=n)
            nc.sync.dma_start(out=w_sb[n][:], in_=t_w[n][:])
        idx_sb = []
        for r in range(2):
            it = cpool.tile([P, TBs[r] * 8], I16, name=f"idxsb{r}", tag=f"idxsb{r}")
            nc.sync.dma_start(out=it[:], in_=t_idx[r][:])
            idx_sb.append(it)
        # persistent per-shard state
        q_sb = [cpool.tile([P, NGRP, C], BF, name=f"qsb{t}", tag=f"qsb{t}")
                for t in range(2)]
        x_sb = [[cpool.tile([P, NGRP, C], BF, name=f"xsb{t}{l}", tag=f"xsb{t}{l}")
                 for l in range(2)] for t in range(2)]
        acc_sb = [cpool.tile([P, NGRP, 132], FP, name=f"accsb{t}", tag=f"accsb{t}")
                  for t in range(2)]
        xsT_sb = {}
        for t, (ap_, din) in enumerate([(t_xasT, DA + 1), (t_xbsT, DB + 1)]):
            xt = cpool.tile([din, SHARD], BF, name=f"xsT{t}", tag=f"xsT{t}")
            nc.sync.dma_start(out=xt[:], in_=ap_[:])
            xsT_sb[t] = xt

        def ag(t, l):
            if os.environ.get("SKIP_AG"):
                for k in range(NCORES):
                    nc.sync.dma_start(out=t_tab[t][l].ap()[k, :, :],
                                      in_=t_agsrc[t][l].ap()[:])
            else:
                nc.gpsimd.collective_compute(
                    "AllGather", mybir.AluOpType.bypass,
                    replica_groups=[list(range(NCORES))],
                    ins=[t_agsrc[t][l].ap()[:]], outs=[t_tab[t][l].ap()[:]],
                )

        # ---------- phase 1: layer-0 own-shard projections ----------
        def phase1(t):
            Win = "Wina" if t == 0 else "Winb"
            din = (DA if t == 0 else DB) + 1
            with (
                tc.tile_pool(name=f"p1s{t}", bufs=3) as sp,
                tc.tile_pool(name=f"p1p{t}", bufs=2, space="PSUM") as pp,
            ):
                for g in range(NGRP):
                    ps0 = pp.tile([P, C], FP, space="PSUM", tag="c1")
                    nc.tensor.matmul(out=ps0[:], lhsT=xsT_sb[t][:, g * P:(g + 1) * P],
                                     rhs=w_sb[Win][:], start=True, stop=True)
                    # relu + cast into resident x0
                    nc.scalar.activation(out=x_sb[t][0][:, g, :], in_=ps0[:], func=AF.Relu)
                    pst = pp.tile([P, P], BF, space="PSUM", tag="pst")
                    nc.tensor.transpose(out=pst[:], in_=x_sb[t][0][:, g, :], identity=ident[:])
                    x0T = sp.tile([P, P], BF, tag="x0T")
                    nc.scalar.activation(out=x0T[:], in_=pst[:], func=AF.Copy)
                    pkv = pp.tile([P, 2 * C], FP, space="PSUM", tag="c2")
                    nc.tensor.matmul(out=pkv[:], lhsT=x0T[:], rhs=w_sb[f"Wkv0{t}"][:],
                                     start=True, stop=True)
                    kvt = sp.tile([P, 2 * C], BF, tag="kvt")
                    nc.scalar.activation(out=kvt[:], in_=pkv[:], func=AF.Copy)
                    nc.sync.dma_start(out=t_agsrc[t][0].ap()[g * P:(g + 1) * P, :], in_=kvt[:])
                    pq = pp.tile([P, C], FP, space="PSUM", tag="c1")
                    nc.tensor.matmul(out=pq[:], lhsT=x0T[:], rhs=w_sb[f"Wq0{t}"][:],
                                     start=True, stop=True)
                    nc.vector.tensor_tensor(out=q_sb[t][:, g, :], in0=pq[:],
                                            in1=w_sb[f"Bq0{t}"][:], op=AL.add)

        # ---------- attention ----------
        def attention(r, l, mid_cb=None):
            """relation r: src type r, dst type 1-r; fills acc_sb[1-r]."""
            sc = scheds[r]
            td = 1 - r
            tabflat = t_tab[r][l].ap().rearrange("k n c -> (k n) c")
            qt = q_sb[td]
            idxt = idx_sb[r]
            blk_grp = sc["blk_grp"]
            with (
                tc.tile_pool(name=f"gat{r}{l}", bufs=2) as gpool,
                tc.tile_pool(name=f"bat{r}{l}", bufs=3) as bpool,
                tc.tile_pool(name=f"aps{r}{l}", bufs=2, space="PSUM") as aps,
                tc.tile_pool(name=f"accp{r}{l}", bufs=2, space="PSUM") as accp,
            ):
                for region in range(2):
                    if region == 1 and mid_cb is not None:
                        mid_cb()
                    r0 = 0 if region == 0 else sc["TLp"]
                    r1 = sc["TLp"] if region == 0 else sc["TB"]
                    nblk_reg = r1 - r0
                    if nblk_reg == 0:
                        continue
                    in_ap = tabflat[0:LO_LIM, :] if region == 0 else tabflat[LO_LIM:NPAD, :]
                    accn = accd = None
                    cur_grp = -1
                    for c0 in range(r0, r1, CHUNK_BLK):
                        n = min(CHUNK_BLK, r1 - c0)
                        gt = gpool.tile([P, CHUNK_BLK, 2 * C], BF, tag="kvchunk")
                        if os.environ.get("SKIP_GATHER"):
                            nc.vector.memset(gt[:, 0:n, :], 1.0)
                        else:
                            nc.gpsimd.dma_gather(
                                out_ap=gt[:, 0:n, :], in_ap=in_ap,
                                idxs_ap=idxt[:, c0 * 8:(c0 + n) * 8],
                                num_idxs=n * P, num_idxs_reg=n * P,
                                elem_size=2 * C, single_packet=False,
                            )
                        oht_c = gpool.tile([P, CHUNK_BLK, P], BF, tag="ohTchunk")
                        nc.sync.dma_start(
                            out=oht_c[:, 0:n, :].rearrange("p a b -> p (a b)"),
                            in_=t_ohT[r][:, c0 * P:(c0 + n) * P])
                        oh_c = gpool.tile([P, CHUNK_BLK, P], BF, tag="ohchunk")
                        nc.sync.dma_start(
                            out=oh_c[:, 0:n, :].rearrange("p a b -> p (a b)"),
                            in_=t_oh[r][:, c0 * P:(c0 + n) * P])
                        for b0 in range(0, n, B):
                            nb = min(B, n - b0)
                            qg_ps = aps.tile([P, B, C], FP, space="PSUM", tag="qg")
                            for i in range(nb):
                                g = blk_grp[c0 + b0 + i]
                                nc.tensor.matmul(out=qg_ps[:, i, :],
                                                 lhsT=oht_c[:, b0 + i, :],
                                                 rhs=qt[:, g, :], start=True, stop=True)
                            qg = bpool.tile([P, B, C], BF, tag="qg_sb")
                            nc.scalar.activation(out=qg[:, 0:nb, :], in_=qg_ps[:, 0:nb, :],
                                                 func=AF.Copy)
                            lp = bpool.tile([P, B, C], BF, tag="lp")
                            nc.vector.tensor_tensor(out=lp[:, 0:nb, :], in0=qg[:, 0:nb, :],
                                                    in1=gt[:, b0:b0 + nb, 0:C], op=AL.mult)
                            z = bpool.tile([P, B * H], FP, tag="z")
                            nc.vector.tensor_reduce(
                                out=z[:, 0:nb * H],
                                in_=lp[:, 0:nb, :].rearrange("p b (h d) -> p (b h) d", h=H),
                                axis=mybir.AxisListType.X, op=AL.add)
                            ze = bpool.tile([P, B * H], BF, tag="ze")
                            nc.scalar.activation(out=ze[:, 0:nb * H], in_=z[:, 0:nb * H],
                                                 func=AF.Exp)
                            zx = bpool.tile([P, B, C], BF, tag="zx")
                            nc.scalar.activation(
                                out=zx[:, 0:nb, :].rearrange("p b (h d) -> p b h d", h=H),
                                in_=ze[:, 0:nb * H].rearrange("p (b h) -> p b h ()", h=H)
                                    .to_broadcast([P, nb, H, D]),
                                func=AF.Copy)
                            wz = bpool.tile([P, B, C], BF, tag="wz")
                            nc.vector.tensor_tensor(out=wz[:, 0:nb, :],
                                                    in0=gt[:, b0:b0 + nb, C:2 * C],
                                                    in1=zx[:, 0:nb, :], op=AL.mult)
                            for i in range(nb):
                                blk = c0 + b0 + i
                                g = blk_grp[blk]
                                if g != cur_grp:
                                    accn = accp.tile([P, C], FP, space="PSUM", tag="accn")
                                    accd = accp.tile([P, H], FP, space="PSUM", tag="accd")
                                    cur_grp = g
                                off = sc["lo_off"][g] if region == 0 else sc["hi_off"][g]
                                cnt = sc["nlo"][g] if region == 0 else sc["nhi"][g]
                                end = off + cnt
                                if g == NGRP - 1:
                                    end = r1    # pads attach to last group
                                first = blk == off
                                last = blk == end - 1
                                nc.tensor.matmul(out=accn[:], lhsT=oh_c[:, b0 + i, :],
                                                 rhs=wz[:, i, :], start=first, stop=last)
                                nc.tensor.matmul(out=accd[:], lhsT=oh_c[:, b0 + i, :],
                                                 rhs=ze[:, i * H:(i + 1) * H], start=first, stop=last)
                                if last:
                                    if region == 0:
                                        nc.scalar.activation(out=acc_sb[td][:, g, 0:C],
                                                             in_=accn[:], func=AF.Copy)
                                        nc.scalar.activation(out=acc_sb[td][:, g, C:C + H],
                                                             in_=accd[:], func=AF.Copy)
                                    else:
                                        nc.vector.tensor_tensor(
                                            out=acc_sb[td][:, g, 0:C], in0=accn[:],
                                            in1=acc_sb[td][:, g, 0:C], op=AL.add)
                                        nc.vector.tensor_tensor(
                                            out=acc_sb[td][:, g, C:C + H], in0=accd[:],
                                            in1=acc_sb[td][:, g, C:C + H], op=AL.add)

        # ---------- alin ----------
        def alin(t, l):
            """a_lin + skip for dst type t, layer l; reads acc_sb[t]. For l=0
            also produces layer-1 q, resident x1, and the layer-1 K|V shard."""
            with (
                tc.tile_pool(name=f"al{t}{l}", bufs=3) as sp,
                tc.tile_pool(name=f"alp{t}{l}", bufs=1, space="PSUM") as pp,
            ):
                for g in range(NGRP):
                    den = sp.tile([P, H], FP, tag="den")
                    nc.vector.tensor_scalar(out=den[:], in0=acc_sb[t][:, g, C:C + H],
                                            scalar1=1e-16, scalar2=None, op0=AL.add)
                    rec = sp.tile([P, H], FP, tag="rec")
                    nc.vector.reciprocal(rec[:], den[:])
                    at = sp.tile([P, C], BF, tag="at")
                    nc.vector.tensor_tensor(
                        out=at[:], in0=acc_sb[t][:, g, 0:C],
                        in1=rec[:].rearrange("p (h o) -> p h o", o=1).to_broadcast([P, H, D]),
                        op=AL.mult)
                    at2 = sp.tile([P, C], BF, tag="at2")
                    nc.vector.tensor_tensor(out=at2[:], in0=at[:], in1=w_sb[f"Bv{l}{t}"][:],
                                            op=AL.add)
                    gl = sp.tile([P, C], BF, tag="gl")
                    nc.scalar.activation(out=gl[:], in_=at2[:], func=AF.Gelu)
                    pst = pp.tile([P, P], BF, space="PSUM", tag="trans")
                    nc.tensor.transpose(out=pst[:], in_=gl[:], identity=ident[:])
                    glT = sp.tile([P, P], BF, tag="glT")
                    nc.scalar.activation(out=glT[:], in_=pst[:], func=AF.Copy)
                    pso = pp.tile([P, C], FP, space="PSUM", tag="c1")
                    nc.tensor.matmul(out=pso[:], lhsT=glT[:], rhs=w_sb[f"Wal{l}{t}"][:],
                                     start=True, stop=True)
                    o1 = sp.tile([P, C], BF, tag="o1")
                    nc.vector.tensor_tensor(out=o1[:], in0=pso[:], in1=w_sb[f"Bal{l}{t}"][:],
                                            op=AL.add)
                    o2 = sp.tile([P, C], BF, tag="o2")
                    nc.scalar.activation(out=o2[:], in_=x_sb[t][l][:, g, :],
                                         func=AF.Copy, scale=float(f[f"oms{l}{t}"]))
                    if l == 0:
                        nw = x_sb[t][1][:, g, :]
                        nc.vector.tensor_tensor(out=nw, in0=o1[:], in1=o2[:], op=AL.add)
                        pst2 = pp.tile([P, P], BF, space="PSUM", tag="trans")
                        nc.tensor.transpose(out=pst2[:], in_=nw, identity=ident[:])
                        nwT = sp.tile([P, P], BF, tag="nwT")
                        nc.scalar.activation(out=nwT[:], in_=pst2[:], func=AF.Copy)
                        pkv = pp.tile([P, 2 * C], FP, space="PSUM", tag="c2")
                        nc.tensor.matmul(out=pkv[:], lhsT=nwT[:], rhs=w_sb[f"Wkv1{t}"][:],
                                         start=True, stop=True)
                        kvt = sp.tile([P, 2 * C], BF, tag="alkvt")
                        nc.scalar.activation(out=kvt[:], in_=pkv[:], func=AF.Copy)
                        nc.sync.dma_start(out=t_agsrc[t][1].ap()[g * P:(g + 1) * P, :],
                                          in_=kvt[:])
                        pq = pp.tile([P, C], FP, space="PSUM", tag="c1")
                        nc.tensor.matmul(out=pq[:], lhsT=nwT[:], rhs=w_sb[f"Wq1{t}"][:],
                                         start=True, stop=True)
                        nc.vector.tensor_tensor(out=q_sb[t][:, g, :], in0=pq[:],
                                                in1=w_sb[f"Bq1{t}"][:], op=AL.add)
                    else:
                        nw = sp.tile([P, C], FP, tag="nwf")
                        nc.vector.tensor_tensor(out=nw[:], in0=o1[:], in1=o2[:], op=AL.add)
                        nc.sync.dma_start(out=t_out[t][g * P:(g + 1) * P, :], in_=nw[:])

        # ---------- schedule ----------
        phase1(0)
        ag(0, 0)
        phase1(1)
        ag(1, 0)
        attention(0, 0)     # dst b, table a/l0
        alin(1, 0)          # x1_b, kv_b_l1 shard
        attention(1, 0, mid_cb=lambda: ag(1, 1))   # dst a, table b/l0
        alin(0, 0)          # x1_a, kv_a_l1 shard
        attention(1, 1, mid_cb=lambda: ag(0, 1))   # dst a, table b/l1
        alin(0, 1)          # out a
        attention(0, 1)     # dst b, table a/l1
        alin(1, 1)          # out b
        cpool_cm.__exit__(None, None, None)

    nc.compile()

    in_maps = []
    for c in range(NCORES):
        m = {"xasT": np.ascontiguousarray(xaT[:, c * SHARD:(c + 1) * SHARD]),
             "xbsT": np.ascontiguousarray(xbT[:, c * SHARD:(c + 1) * SHARD]),
             "idx0": idx0[c], "oh0": oh0[c], "ohT0": ohT0[c],
             "idx1": idx1[c], "oh1": oh1[c], "ohT1": ohT1[c]}
        for n in wnames:
            m[n] = np.ascontiguousarray(f[n])
        in_maps.append(m)

    res = run_bass_kernel_spmd(
        nc, in_maps, core_ids=list(range(NCORES)),
        trace=bool(os.environ.get("BASS_TRACE")),
    )
    LAST_RESULT = res
    outa = np.concatenate([res.results[c]["out0"] for c in range(NCORES)])[:40000]
    outb = np.concatenate([res.results[c]["out1"] for c in range(NCORES)])[:40000]
    return outa, outb


# revision 9
# speedup vs baseline: 2.8554x; 1.2497x over previous
"""HGT (2-type, 2-relation, 2-layer) Bass kernel for 8 Trainium2 cores — v2.

Sharding: destination-node sharding; core c owns dst rows [5120c, 5120(c+1))
of both node types. bf16 on-chip pipeline with fp32 PSUM accumulation.

Key structure vs v1:
- Own-shard K/V projection only; full K/V tables assembled via AllGather of
  bf16 shards (per source-type per layer). K bias dropped (cancels in the
  per-dst softmax); V bias applied after normalization, before gelu.
- Per-edge gather of combined K|V rows (512B bf16) with int16 indices,
  lo/hi split at 32768.
- One-hot (oh: [edge,dst], ohT: [dst,edge]) blocks precomputed on host,
  streamed from DRAM as bf16; per-128-edge-block matmuls do the q gather
  (lhsT=ohT) and the segment-sum scatter (lhsT=oh) with fp32 PSUM accum.
- Vector work batched 4 blocks per instruction; exp/copies on the scalar
  (ACT) engine; input-proj bias folded into the matmul via a ones row.
"""
import math
import os
import sys

import numpy as np

sys.path.insert(0, "/opt/trn_rl_repo")

import ml_dtypes

BF16 = ml_dtypes.bfloat16

H, D, C, L = 4, 32, 128, 2
INV_SQRT_D = 1.0 / math.sqrt(D)
P = 128
NCORES = 8
SHARD = 5120
NGRP = SHARD // P     # 40
NPAD = NCORES * SHARD # 40960
LO_LIM = 32768
HI_ROWS = NPAD - LO_LIM
CHUNK_BLK = 16        # gather chunk size in 128-edge blocks (multiple of 4)
B = 4                 # vector batch size in blocks

LAST_RESULT = None


def _ceil4(x):
    return (x + 3) // 4 * 4


def _fold_weights(ins):
    """Fold a_rel/m_rel into k/v weights, p_rel/sqrt(D) into q, sigmoid(skip)
    into a_lin. K bias dropped (softmax-invariant); V bias kept separately
    (applied post-normalization). Returns dict of numpy arrays (bf16)."""
    f = {}
    for l in range(L):
        for t in range(2):
            kw = np.asarray(ins["k_w"][l, t], np.float32)   # [C, C]
            kb = np.asarray(ins["k_b"][l, t], np.float32)
            vw = np.asarray(ins["v_w"][l, t], np.float32)
            vb = np.asarray(ins["v_b"][l, t], np.float32)
            ar = np.asarray(ins["a_rel"][l, t], np.float32)  # [H, D, D]
            mr = np.asarray(ins["m_rel"][l, t], np.float32)
            wk = np.zeros((C, C), np.float32)
            wv = np.zeros((C, C), np.float32)
            bv = np.zeros(C, np.float32)
            for h in range(H):
                sl = slice(h * D, (h + 1) * D)
                wk[:, sl] = kw[:, sl] @ ar[h]
                wv[:, sl] = vw[:, sl] @ mr[h]
                bv[sl] = vb[sl] @ mr[h]
            del kb
            f[f"Wkv{l}{t}"] = np.concatenate([wk, wv], axis=1).astype(BF16)  # [C,2C]
            # relation t's dst type is 1-t: bv applied in alin(1-t, l)
            f[f"Bv{l}{1 - t}"] = np.tile(bv[None, :], (P, 1)).astype(BF16)
            r_dst = 1 - t
            pr = np.asarray(ins["p_rel"][l, r_dst], np.float32) * INV_SQRT_D
            scale = np.repeat(pr, D)
            f[f"Wq{l}{t}"] = (np.asarray(ins["q_w"][l, t], np.float32) * scale[None, :]).astype(BF16)
            f[f"Bq{l}{t}"] = np.tile((np.asarray(ins["q_b"][l, t], np.float32) * scale)[None, :], (P, 1)).astype(BF16)
            s = 1.0 / (1.0 + math.exp(-float(np.asarray(ins["skip"][l, t]))))
            f[f"Wal{l}{t}"] = (np.asarray(ins["a_lin_w"][l, t], np.float32) * s).astype(BF16)
            f[f"Bal{l}{t}"] = np.tile((np.asarray(ins["a_lin_b"][l, t], np.float32) * s)[None, :], (P, 1)).astype(BF16)
            f[f"oms{l}{t}"] = 1.0 - s
    # input linears with folded bias row (ones appended to lhsT on host)
    wina = np.asarray(ins["lin_a_w"], np.float32)   # [64, C]
    bina = np.asarray(ins["lin_a_b"], np.float32)
    winb = np.asarray(ins["lin_b_w"], np.float32)   # [32, C]
    binb = np.asarray(ins["lin_b_b"], np.float32)
    f["Wina"] = np.concatenate([wina, bina[None, :]], 0).astype(BF16)  # [65, C]
    f["Winb"] = np.concatenate([winb, binb[None, :]], 0).astype(BF16)  # [33, C]
    return f


def _prep_edges(edge):
    """Partition one relation's edges by dst shard. Returns
    (idx_w[8], oh[8], ohT[8], sched) where sched describes the shared static
    block schedule: dict with nlo, nhi, TLp, THp, TB, and per-block
    (group, first, last) info per region."""
    src = np.asarray(edge[0]).astype(np.int64)
    dst = np.asarray(edge[1]).astype(np.int64)
    core = dst // SHARD
    nlo = np.zeros(NGRP, np.int64)
    nhi = np.zeros(NGRP, np.int64)
    percore = []
    for c in range(NCORES):
        m = core == c
        s, dl = src[m], dst[m] - c * SHARD
        g = dl // P
        rel = dl % P
        lo = s < LO_LIM
        percore.append((s, g, rel, lo))
        for gi in range(NGRP):
            gm = g == gi
            nlo[gi] = max(nlo[gi], int(np.sum(gm & lo)))
            nhi[gi] = max(nhi[gi], int(np.sum(gm & ~lo)))
    nlo = np.maximum((nlo + P - 1) // P, 1)           # blocks per group, >=1
    nhi = (nhi + P - 1) // P
    TL, TH = int(nlo.sum()), int(nhi.sum())
    TLp, THp = _ceil4(TL), _ceil4(TH)
    TB = TLp + THp
    lo_off = np.concatenate([[0], np.cumsum(nlo)[:-1]])
    hi_off = np.concatenate([[0], np.cumsum(nhi)[:-1]]) + TLp

    # per-block group assignment (pads attach to last group)
    blk_grp = np.zeros(TB, np.int64)
    for gi in range(NGRP):
        blk_grp[lo_off[gi]:lo_off[gi] + nlo[gi]] = gi
        blk_grp[hi_off[gi]:hi_off[gi] + nhi[gi]] = gi
    blk_grp[TL:TLp] = NGRP - 1
    blk_grp[TLp + TH:TB] = NGRP - 1

    idx_ws, ohs, ohTs = [], [], []
    for c in range(NCORES):
        s, g, rel, lo = percore[c]
        idx = np.zeros(TB * P, np.int16)
        dr = np.full(TB * P, -1.0, np.float32)
        for gi in range(NGRP):
            for reg, off in ((True, lo_off[gi]), (False, hi_off[gi])):
                gm = (g == gi) & (lo == reg)
                sg, rg = s[gm], rel[gm]
                o = np.argsort(sg, kind="stable")
                sg, rg = sg[o], rg[o]
                base = int(off) * P
                idx[base:base + len(sg)] = (sg if reg else sg - LO_LIM).astype(np.int16)
                dr[base:base + len(sg)] = rg
        idx_ws.append(np.tile(idx.reshape(TB * P // 16, 16).T, (8, 1)).copy())
        drb = dr.reshape(TB, P)                                   # [blk, e]
        j = np.arange(P, dtype=np.float32)
        oh = (drb[:, :, None] == j[None, None, :])                # [blk, e, j]
        ohs.append(np.ascontiguousarray(
            oh.transpose(1, 0, 2).reshape(P, TB * P)).astype(BF16))
        ohT = (drb[:, None, :] == j[None, :, None])               # [blk, j, e]
        ohTs.append(np.ascontiguousarray(
            ohT.transpose(1, 0, 2).reshape(P, TB * P)).astype(BF16))
    sched = dict(nlo=nlo.tolist(), nhi=nhi.tolist(),
                 lo_off=lo_off.tolist(), hi_off=hi_off.tolist(),
                 TL=TL, TH=TH, TLp=TLp, THp=THp, TB=TB,
                 blk_grp=blk_grp.tolist())
    return idx_ws, ohs, ohTs, sched


def kernel(**ins):
    global LAST_RESULT
    import concourse.bass as bass
    import concourse.tile as tile
    from concourse import bacc, mybir
    from concourse.bass_utils import run_bass_kernel_spmd

    FP = mybir.dt.float32
    BF = mybir.dt.bfloat16
    I16 = mybir.dt.int16
    AL = mybir.AluOpType
    AF = mybir.ActivationFunctionType

    f = _fold_weights(ins)
    idx0, oh0, ohT0, sc0 = _prep_edges(np.asarray(ins["edge_ab"]))
    idx1, oh1, ohT1, sc1 = _prep_edges(np.asarray(ins["edge_ba"]))
    scheds = [sc0, sc1]
    TBs = [sc0["TB"], sc1["TB"]]

    xa = np.asarray(ins["x_a"], np.float32)
    xb = np.asarray(ins["x_b"], np.float32)
    DA, DB = xa.shape[1], xb.shape[1]
    # transposed, zero-padded, ones row appended (bias fold), bf16
    xaT = np.zeros((DA + 1, NPAD), np.float32)
    xaT[:DA, :40000] = xa.T
    xaT[DA, :] = 1.0
    xbT = np.zeros((DB + 1, NPAD), np.float32)
    xbT[:DB, :40000] = xb.T
    xbT[DB, :] = 1.0
    xaT = xaT.astype(BF16)
    xbT = xbT.astype(BF16)

    nc = bacc.Bacc("TRN2", target_bir_lowering=False, debug=False, num_devices=NCORES)

    # ---- DRAM tensors ----
    t_xasT = nc.dram_tensor("xasT", [DA + 1, SHARD], BF, kind="ExternalInput").ap()
    t_xbsT = nc.dram_tensor("xbsT", [DB + 1, SHARD], BF, kind="ExternalInput").ap()
    wnames = ["Wina", "Winb"]
    for l in range(L):
        for t in range(2):
            wnames += [f"Wkv{l}{t}", f"Wq{l}{t}", f"Bq{l}{t}",
                       f"Wal{l}{t}", f"Bal{l}{t}", f"Bv{l}{t}"]
    t_w = {n: nc.dram_tensor(n, list(f[n].shape), BF, kind="ExternalInput").ap()
           for n in wnames}
    t_idx = [nc.dram_tensor(f"idx{r}", [P, TBs[r] * 8], I16, kind="ExternalInput").ap()
             for r in range(2)]
    t_oh = [nc.dram_tensor(f"oh{r}", [P, TBs[r] * P], BF, kind="ExternalInput").ap()
            for r in range(2)]
    t_ohT = [nc.dram_tensor(f"ohT{r}", [P, TBs[r] * P], BF, kind="ExternalInput").ap()
             for r in range(2)]

    # K|V tables per (src type, layer): AllGather output, viewed flat for gathers
    t_tab = [[nc.dram_tensor(f"tab{t}{l}", [NCORES, SHARD, 2 * C], BF,
                             addr_space="Shared") for l in range(L)]
             for t in range(2)]
    t_agsrc = [[nc.dram_tensor(f"agsrc{t}{l}", [SHARD, 2 * C], BF)
                for l in range(L)] for t in range(2)]
    t_out = [nc.dram_tensor(f"out{t}", [SHARD, C], FP, kind="ExternalOutput").ap()
             for t in range(2)]

    with tile.TileContext(nc) as tc:
        cpool_cm = tc.tile_pool(name="const", bufs=1)
        cpool = cpool_cm.__enter__()
        ident = cpool.tile([P, P], BF)
        from concourse.masks import make_identity
        make_identity(nc, ident[:])
        w_sb = {}
        for n in wnames:
            w_sb[n] = cpool.tile(list(f[n].shape), BF, name=n, tag=n)
            nc.sync.dma_start(out=w_sb[n][:], in_=t_w[n][:])
        idx_sb = []
        for r in range(2):
            it = cpool.tile([P, TBs[r] * 8], I16, name=f"idxsb{r}", tag=f"idxsb{r}")
            nc.sync.dma_start(out=it[:], in_=t_idx[r][:])
            idx_sb.append(it)
        # persistent per-shard state
        q_sb = [cpool.tile([P, NGRP, C], BF, name=f"qsb{t}", tag=f"qsb{t}")
                for t in range(2)]
        x_sb = [[cpool.tile([P, NGRP, C], BF, name=f"xsb{t}{l}", tag=f"xsb{t}{l}")
                 for l in range(2)] for t in range(2)]
        acc_sb = [cpool.tile([P, NGRP, 132], FP, name=f"accsb{t}", tag=f"accsb{t}")
                  for t in range(2)]
        xsT_sb = {}
        for t, (ap_, din) in enumerate([(t_xasT, DA + 1), (t_xbsT, DB + 1)]):
            xt = cpool.tile([din, SHARD], BF, name=f"xsT{t}", tag=f"xsT{t}")
            nc.sync.dma_start(out=xt[:], in_=ap_[:])
            xsT_sb[t] = xt

        def ag(t, l):
            if os.environ.get("SKIP_AG"):
                for k in range(NCORES):
                    nc.sync.dma_start(out=t_tab[t][l].ap()[k, :, :],
                                      in_=t_agsrc[t][l].ap()[:])
            else:
                nc.gpsimd.collective_compute(
                    "AllGather", mybir.AluOpType.bypass,
                    replica_groups=[list(range(NCORES))],
                    ins=[t_agsrc[t][l].ap()[:]], outs=[t_tab[t][l].ap()[:]],
                )

        # ---------- phase 1: layer-0 own-shard projections ----------
        def phase1(t):
            Win = "Wina" if t == 0 else "Winb"
            din = (DA if t == 0 else DB) + 1
            with (
                tc.tile_pool(name=f"p1s{t}", bufs=3) as sp,
                tc.tile_pool(name=f"p1p{t}", bufs=1, space="PSUM") as pp,
            ):
                for g in range(NGRP):
                    ps0 = pp.tile([P, C], FP, space="PSUM", tag="c1")
                    nc.tensor.matmul(out=ps0[:], lhsT=xsT_sb[t][:, g * P:(g + 1) * P],
                                     rhs=w_sb[Win][:], start=True, stop=True)
                    # relu + cast into resident x0
                    nc.scalar.activation(out=x_sb[t][0][:, g, :], in_=ps0[:], func=AF.Relu)
                    pst = pp.tile([P, P], BF, space="PSUM", tag="pst")
                    nc.tensor.transpose(out=pst[:], in_=x_sb[t][0][:, g, :], identity=ident[:])
                    x0T = sp.tile([P, P], BF, tag="x0T")
                    nc.scalar.activation(out=x0T[:], in_=pst[:], func=AF.Copy)
                    pkv = pp.tile([P, 2 * C], FP, space="PSUM", tag="c2")
                    nc.tensor.matmul(out=pkv[:], lhsT=x0T[:], rhs=w_sb[f"Wkv0{t}"][:],
                                     start=True, stop=True)
                    kvt = sp.tile([P, 2 * C], BF, tag="kvt")
                    nc.scalar.activation(out=kvt[:], in_=pkv[:], func=AF.Copy)
                    nc.sync.dma_start(out=t_agsrc[t][0].ap()[g * P:(g + 1) * P, :], in_=kvt[:])
                    pq = pp.tile([P, C], FP, space="PSUM", tag="c1")
                    nc.tensor.matmul(out=pq[:], lhsT=x0T[:], rhs=w_sb[f"Wq0{t}"][:],
                                     start=True, stop=True)
                    nc.vector.tensor_tensor(out=q_sb[t][:, g, :], in0=pq[:],
                                            in1=w_sb[f"Bq0{t}"][:], op=AL.add)

        # ---------- attention ----------
        gpool_cm = tc.tile_pool(name="gat", bufs=3)
        gpool = gpool_cm.__enter__()
        bpool_cm = tc.tile_pool(name="bat", bufs=3)
        bpool = bpool_cm.__enter__()
        aps_cm = tc.tile_pool(name="aps", bufs=2, space="PSUM")
        aps = aps_cm.__enter__()
        accp_cm = tc.tile_pool(name="accp", bufs=2, space="PSUM")
        accp = accp_cm.__enter__()

        def attention(r, l, mid_cb=None):
            """relation r: src type r, dst type 1-r; fills acc_sb[1-r]."""
            sc = scheds[r]
            td = 1 - r
            tabflat = t_tab[r][l].ap().rearrange("k n c -> (k n) c")
            qt = q_sb[td]
            idxt = idx_sb[r]
            blk_grp = sc["blk_grp"]
            if True:
                for region in range(2):
                    if region == 1 and mid_cb is not None:
                        mid_cb()
                    r0 = 0 if region == 0 else sc["TLp"]
                    r1 = sc["TLp"] if region == 0 else sc["TB"]
                    nblk_reg = r1 - r0
                    if nblk_reg == 0:
                        continue
                    in_ap = tabflat[0:LO_LIM, :] if region == 0 else tabflat[LO_LIM:NPAD, :]
                    accps = None
                    cur_grp = -1
                    for c0 in range(r0, r1, CHUNK_BLK):
                        n = min(CHUNK_BLK, r1 - c0)
                        gt = gpool.tile([P, CHUNK_BLK, 2 * C], BF, tag="kvchunk")
                        if os.environ.get("SKIP_GATHER"):
                            nc.vector.memset(gt[:, 0:n, :], 1.0)
                        else:
                            nc.gpsimd.dma_gather(
                                out_ap=gt[:, 0:n, :], in_ap=in_ap,
                                idxs_ap=idxt[:, c0 * 8:(c0 + n) * 8],
                                num_idxs=n * P, num_idxs_reg=n * P,
                                elem_size=2 * C, single_packet=False,
                            )
                        oht_c = gpool.tile([P, CHUNK_BLK, P], BF, tag="ohTchunk")
                        nc.sync.dma_start(
                            out=oht_c[:, 0:n, :].rearrange("p a b -> p (a b)"),
                            in_=t_ohT[r][:, c0 * P:(c0 + n) * P])
                        oh_c = gpool.tile([P, CHUNK_BLK, P], BF, tag="ohchunk")
                        nc.sync.dma_start(
                            out=oh_c[:, 0:n, :].rearrange("p a b -> p (a b)"),
                            in_=t_oh[r][:, c0 * P:(c0 + n) * P])
                        for b0 in range(0, n, B):
                            nb = min(B, n - b0)
                            qg_ps = aps.tile([P, B, C], FP, space="PSUM", tag="qg")
                            for i in range(nb):
                                g = blk_grp[c0 + b0 + i]
                                nc.tensor.matmul(out=qg_ps[:, i, :],
                                                 lhsT=oht_c[:, b0 + i, :],
                                                 rhs=qt[:, g, :], start=True, stop=True)
                            qg = bpool.tile([P, B, C], BF, tag="qg_sb")
                            nc.scalar.activation(out=qg[:, 0:nb, :], in_=qg_ps[:, 0:nb, :],
                                                 func=AF.Copy)
                            lp = bpool.tile([P, B, C], BF, tag="lp")
                            nc.vector.tensor_tensor(out=lp[:, 0:nb, :], in0=qg[:, 0:nb, :],
                                                    in1=gt[:, b0:b0 + nb, 0:C], op=AL.mult)
                            z = bpool.tile([P, B * H], FP, tag="z")
                            nc.vector.tensor_reduce(
                                out=z[:, 0:nb * H],
                                in_=lp[:, 0:nb, :].rearrange("p b (h d) -> p (b h) d", h=H),
                                axis=mybir.AxisListType.X, op=AL.add)
                            ze = bpool.tile([P, B * H], BF, tag="ze")
                            nc.scalar.activation(out=ze[:, 0:nb * H], in_=z[:, 0:nb * H],
                                                 func=AF.Exp)
                            zx = bpool.tile([P, B, C], BF, tag="zx")
                            nc.scalar.activation(
                                out=zx[:, 0:nb, :].rearrange("p b (h d) -> p b h d", h=H),
                                in_=ze[:, 0:nb * H].rearrange("p (b h) -> p b h ()", h=H)
                                    .to_broadcast([P, nb, H, D]),
                                func=AF.Copy)
                            wz = bpool.tile([P, B, 132], BF, tag="wz")
                            nc.vector.tensor_tensor(out=wz[:, 0:nb, 0:C],
                                                    in0=gt[:, b0:b0 + nb, C:2 * C],
                                                    in1=zx[:, 0:nb, :], op=AL.mult)
                            nc.scalar.activation(
                                out=wz[:, 0:nb, C:C + H],
                                in_=ze[:, 0:nb * H].rearrange("p (b h) -> p b h", h=H),
                                func=AF.Copy)
                            for i in range(nb):
                                blk = c0 + b0 + i
                                g = blk_grp[blk]
                                if g != cur_grp:
                                    accps = accp.tile([P, 132], FP, space="PSUM", tag="acc")
                                    cur_grp = g
                                off = sc["lo_off"][g] if region == 0 else sc["hi_off"][g]
                                cnt = sc["nlo"][g] if region == 0 else sc["nhi"][g]
                                end = off + cnt
                                if g == NGRP - 1:
                                    end = r1    # pads attach to last group
                                first = blk == off
                                last = blk == end - 1
                                nc.tensor.matmul(out=accps[:], lhsT=oh_c[:, b0 + i, :],
                                                 rhs=wz[:, i, :], start=first, stop=last)
                                if last:
                                    if region == 0:
                                        nc.scalar.activation(out=acc_sb[td][:, g, :],
                                                             in_=accps[:], func=AF.Copy)
                                    else:
                                        nc.vector.tensor_tensor(
                                            out=acc_sb[td][:, g, :], in0=accps[:],
                                            in1=acc_sb[td][:, g, :], op=AL.add)

        # ---------- alin ----------
        def alin(t, l):
            """a_lin + skip for dst type t, layer l; reads acc_sb[t]. For l=0
            also produces layer-1 q, resident x1, and the layer-1 K|V shard."""
            with (
                tc.tile_pool(name=f"al{t}{l}", bufs=3) as sp,
                tc.tile_pool(name=f"alp{t}{l}", bufs=1, space="PSUM") as pp,
            ):
                for g in range(NGRP):
                    den = sp.tile([P, H], FP, tag="den")
                    nc.vector.tensor_scalar(out=den[:], in0=acc_sb[t][:, g, C:C + H],
                                            scalar1=1e-16, scalar2=None, op0=AL.add)
                    rec = sp.tile([P, H], FP, tag="rec")
                    nc.vector.reciprocal(rec[:], den[:])
                    at = sp.tile([P, C], BF, tag="at")
                    nc.vector.tensor_tensor(
                        out=at[:], in0=acc_sb[t][:, g, 0:C],
                        in1=rec[:].rearrange("p (h o) -> p h o", o=1).to_broadcast([P, H, D]),
                        op=AL.mult)
                    at2 = sp.tile([P, C], BF, tag="at2")
                    nc.vector.tensor_tensor(out=at2[:], in0=at[:], in1=w_sb[f"Bv{l}{t}"][:],
                                            op=AL.add)
                    gl = sp.tile([P, C], BF, tag="gl")
                    nc.scalar.activation(out=gl[:], in_=at2[:], func=AF.Gelu)
                    pst = pp.tile([P, P], BF, space="PSUM", tag="trans")
                    nc.tensor.transpose(out=pst[:], in_=gl[:], identity=ident[:])
                    glT = sp.tile([P, P], BF, tag="glT")
                    nc.scalar.activation(out=glT[:], in_=pst[:], func=AF.Copy)
                    pso = pp.tile([P, C], FP, space="PSUM", tag="c1")
                    nc.tensor.matmul(out=pso[:], lhsT=glT[:], rhs=w_sb[f"Wal{l}{t}"][:],
                                     start=True, stop=True)
                    o1 = sp.tile([P, C], BF, tag="o1")
                    nc.vector.tensor_tensor(out=o1[:], in0=pso[:], in1=w_sb[f"Bal{l}{t}"][:],
                                            op=AL.add)
                    o2 = sp.tile([P, C], BF, tag="o2")
                    nc.scalar.activation(out=o2[:], in_=x_sb[t][l][:, g, :],
                                         func=AF.Copy, scale=float(f[f"oms{l}{t}"]))
                    if l == 0:
                        nw = x_sb[t][1][:, g, :]
                        nc.vector.tensor_tensor(out=nw, in0=o1[:], in1=o2[:], op=AL.add)
                        pst2 = pp.tile([P, P], BF, space="PSUM", tag="trans")
                        nc.tensor.transpose(out=pst2[:], in_=nw, identity=ident[:])
                        nwT = sp.tile([P, P], BF, tag="nwT")
                        nc.scalar.activation(out=nwT[:], in_=pst2[:], func=AF.Copy)
                        pkv = pp.tile([P, 2 * C], FP, space="PSUM", tag="c2")
                        nc.tensor.matmul(out=pkv[:], lhsT=nwT[:], rhs=w_sb[f"Wkv1{t}"][:],
                                         start=True, stop=True)
                        kvt = sp.tile([P, 2 * C], BF, tag="alkvt")
                        nc.scalar.activation(out=kvt[:], in_=pkv[:], func=AF.Copy)
                        nc.sync.dma_start(out=t_agsrc[t][1].ap()[g * P:(g + 1) * P, :],
                                          in_=kvt[:])
                        pq = pp.tile([P, C], FP, space="PSUM", tag="c1")
                        nc.tensor.matmul(out=pq[:], lhsT=nwT[:], rhs=w_sb[f"Wq1{t}"][:],
                                         start=True, stop=True)
                        nc.vector.tensor_tensor(out=q_sb[t][:, g, :], in0=pq[:],
                                                in1=w_sb[f"Bq1{t}"][:], op=AL.add)
                    else:
                        nw = sp.tile([P, C], FP, tag="nwf")
                        nc.vector.tensor_tensor(out=nw[:], in0=o1[:], in1=o2[:], op=AL.add)
                        nc.sync.dma_start(out=t_out[t][g * P:(g + 1) * P, :], in_=nw[:])

        # ---------- schedule ----------
        phase1(0)
        ag(0, 0)
        phase1(1)
        ag(1, 0)
        attention(0, 0)     # dst b, table a/l0
        alin(1, 0)          # x1_b, kv_b_l1 shard
        attention(1, 0, mid_cb=lambda: ag(1, 1))   # dst a, table b/l0
        alin(0, 0)          # x1_a, kv_a_l1 shard
        attention(1, 1, mid_cb=lambda: ag(0, 1))   # dst a, table b/l1
        alin(0, 1)          # out a
        attention(0, 1)     # dst b, table a/l1
        alin(1, 1)          # out b
        accp_cm.__exit__(None, None, None)
        aps_cm.__exit__(None, None, None)
        bpool_cm.__exit__(None, None, None)
        gpool_cm.__exit__(None, None, None)
        cpool_cm.__exit__(None, None, None)

    nc.compile()

    in_maps = []
    for c in range(NCORES):
        m = {"xasT": np.ascontiguousarray(xaT[:, c * SHARD:(c + 1) * SHARD]),
             "xbsT": np.ascontiguousarray(xbT[:, c * SHARD:(c + 1) * SHARD]),
             "idx0": idx0[c], "oh0": oh0[c], "ohT0": ohT0[c],
             "idx1": idx1[c], "oh1": oh1[c], "ohT1": ohT1[c]}
        for n in wnames:
            m[n] = np.ascontiguousarray(f[n])
        in_maps.append(m)

    res = run_bass_kernel_spmd(
        nc, in_maps, core_ids=list(range(NCORES)),
        trace=bool(os.environ.get("BASS_TRACE")),
    )
    LAST_RESULT = res
    outa = np.concatenate([res.results[c]["out0"] for c in range(NCORES)])[:40000]
    outb = np.concatenate([res.results[c]["out1"] for c in range(NCORES)])[:40000]
    return outa, outb


# revision 10
# speedup vs baseline: 3.0804x; 1.0788x over previous
"""HGT (2-type, 2-relation, 2-layer) Bass kernel for 8 Trainium2 cores — v2.

Sharding: destination-node sharding; core c owns dst rows [5120c, 5120(c+1))
of both node types. bf16 on-chip pipeline with fp32 PSUM accumulation.

Key structure vs v1:
- Own-shard K/V projection only; full K/V tables assembled via AllGather of
  bf16 shards (per source-type per layer). K bias dropped (cancels in the
  per-dst softmax); V bias applied after normalization, before gelu.
- Per-edge gather of combined K|V rows (512B bf16) with int16 indices,
  lo/hi split at 32768.
- One-hot (oh: [edge,dst], ohT: [dst,edge]) blocks precomputed on host,
  streamed from DRAM as bf16; per-128-edge-block matmuls do the q gather
  (lhsT=ohT) and the segment-sum scatter (lhsT=oh) with fp32 PSUM accum.
- Vector work batched 4 blocks per instruction; exp/copies on the scalar
  (ACT) engine; input-proj bias folded into the matmul via a ones row.
"""
import math
import os
import sys

import numpy as np

sys.path.insert(0, "/opt/trn_rl_repo")

import ml_dtypes

BF16 = ml_dtypes.bfloat16

H, D, C, L = 4, 32, 128, 2
INV_SQRT_D = 1.0 / math.sqrt(D)
P = 128
NCORES = 8
SHARD = 5120
NGRP = SHARD // P     # 40
NPAD = NCORES * SHARD # 40960
LO_LIM = 32768
HI_ROWS = NPAD - LO_LIM
CHUNK_BLK = 16        # gather chunk size in 128-edge blocks (multiple of 4)
B = 4                 # vector batch size in blocks

LAST_RESULT = None


def _ceil4(x):
    return (x + 3) // 4 * 4


def _fold_weights(ins):
    """Fold a_rel/m_rel into k/v weights, p_rel/sqrt(D) into q, sigmoid(skip)
    into a_lin. K bias dropped (softmax-invariant); V bias kept separately
    (applied post-normalization). Returns dict of numpy arrays (bf16)."""
    f = {}
    for l in range(L):
        for t in range(2):
            kw = np.asarray(ins["k_w"][l, t], np.float32)   # [C, C]
            kb = np.asarray(ins["k_b"][l, t], np.float32)
            vw = np.asarray(ins["v_w"][l, t], np.float32)
            vb = np.asarray(ins["v_b"][l, t], np.float32)
            ar = np.asarray(ins["a_rel"][l, t], np.float32)  # [H, D, D]
            mr = np.asarray(ins["m_rel"][l, t], np.float32)
            wk = np.zeros((C, C), np.float32)
            wv = np.zeros((C, C), np.float32)
            bv = np.zeros(C, np.float32)
            for h in range(H):
                sl = slice(h * D, (h + 1) * D)
                wk[:, sl] = kw[:, sl] @ ar[h]
                wv[:, sl] = vw[:, sl] @ mr[h]
                bv[sl] = vb[sl] @ mr[h]
            del kb
            f[f"Wkv{l}{t}"] = np.concatenate([wk, wv], axis=1).astype(BF16)  # [C,2C]
            # relation t's dst type is 1-t: bv applied in alin(1-t, l)
            f[f"Bv{l}{1 - t}"] = np.tile(bv[None, :], (P, 1)).astype(BF16)
            r_dst = 1 - t
            pr = np.asarray(ins["p_rel"][l, r_dst], np.float32) * INV_SQRT_D
            scale = np.repeat(pr, D)
            f[f"Wq{l}{t}"] = (np.asarray(ins["q_w"][l, t], np.float32) * scale[None, :]).astype(BF16)
            f[f"Wkvq{l}{t}"] = np.concatenate(
                [np.asarray(f[f"Wkv{l}{t}"], np.float32),
                 np.asarray(f[f"Wq{l}{t}"], np.float32)], axis=1).astype(BF16)  # [C,3C]
            f[f"Bq{l}{t}"] = np.tile((np.asarray(ins["q_b"][l, t], np.float32) * scale)[None, :], (P, 1)).astype(BF16)
            s = 1.0 / (1.0 + math.exp(-float(np.asarray(ins["skip"][l, t]))))
            f[f"Wal{l}{t}"] = (np.asarray(ins["a_lin_w"][l, t], np.float32) * s).astype(BF16)
            f[f"Bal{l}{t}"] = np.tile((np.asarray(ins["a_lin_b"][l, t], np.float32) * s)[None, :], (P, 1)).astype(BF16)
            f[f"oms{l}{t}"] = 1.0 - s
    # input linears with folded bias row (ones appended to lhsT on host)
    wina = np.asarray(ins["lin_a_w"], np.float32)   # [64, C]
    bina = np.asarray(ins["lin_a_b"], np.float32)
    winb = np.asarray(ins["lin_b_w"], np.float32)   # [32, C]
    binb = np.asarray(ins["lin_b_b"], np.float32)
    f["Wina"] = np.concatenate([wina, bina[None, :]], 0).astype(BF16)  # [65, C]
    f["Winb"] = np.concatenate([winb, binb[None, :]], 0).astype(BF16)  # [33, C]
    return f


def _prep_edges(edge):
    """Partition one relation's edges by dst shard. Returns
    (idx_w[8], oh[8], ohT[8], sched) where sched describes the shared static
    block schedule: dict with nlo, nhi, TLp, THp, TB, and per-block
    (group, first, last) info per region."""
    src = np.asarray(edge[0]).astype(np.int64)
    dst = np.asarray(edge[1]).astype(np.int64)
    core = dst // SHARD
    nlo = np.zeros(NGRP, np.int64)
    nhi = np.zeros(NGRP, np.int64)
    percore = []
    for c in range(NCORES):
        m = core == c
        s, dl = src[m], dst[m] - c * SHARD
        g = dl // P
        rel = dl % P
        lo = s < LO_LIM
        percore.append((s, g, rel, lo))
        for gi in range(NGRP):
            gm = g == gi
            nlo[gi] = max(nlo[gi], int(np.sum(gm & lo)))
            nhi[gi] = max(nhi[gi], int(np.sum(gm & ~lo)))
    nlo = np.maximum((nlo + P - 1) // P, 1)           # blocks per group, >=1
    nhi = (nhi + P - 1) // P
    TL, TH = int(nlo.sum()), int(nhi.sum())
    TLp, THp = _ceil4(TL), _ceil4(TH)
    TB = TLp + THp
    lo_off = np.concatenate([[0], np.cumsum(nlo)[:-1]])
    hi_off = np.concatenate([[0], np.cumsum(nhi)[:-1]]) + TLp

    # per-block group assignment (pads attach to last group)
    blk_grp = np.zeros(TB, np.int64)
    for gi in range(NGRP):
        blk_grp[lo_off[gi]:lo_off[gi] + nlo[gi]] = gi
        blk_grp[hi_off[gi]:hi_off[gi] + nhi[gi]] = gi
    blk_grp[TL:TLp] = NGRP - 1
    blk_grp[TLp + TH:TB] = NGRP - 1

    idx_ws, ohs, ohTs = [], [], []
    for c in range(NCORES):
        s, g, rel, lo = percore[c]
        idx = np.zeros(TB * P, np.int16)
        dr = np.full(TB * P, -1.0, np.float32)
        for gi in range(NGRP):
            for reg, off in ((True, lo_off[gi]), (False, hi_off[gi])):
                gm = (g == gi) & (lo == reg)
                sg, rg = s[gm], rel[gm]
                o = np.argsort(sg, kind="stable")
                sg, rg = sg[o], rg[o]
                base = int(off) * P
                idx[base:base + len(sg)] = (sg if reg else sg - LO_LIM).astype(np.int16)
                dr[base:base + len(sg)] = rg
        idx_ws.append(np.tile(idx.reshape(TB * P // 16, 16).T, (8, 1)).copy())
        drb = dr.reshape(TB, P)                                   # [blk, e]
        j = np.arange(P, dtype=np.float32)
        oh = (drb[:, :, None] == j[None, None, :])                # [blk, e, j]
        ohs.append(np.ascontiguousarray(
            oh.transpose(1, 0, 2).reshape(P, TB * P)).astype(BF16))
        ohT = (drb[:, None, :] == j[None, :, None])               # [blk, j, e]
        ohTs.append(np.ascontiguousarray(
            ohT.transpose(1, 0, 2).reshape(P, TB * P)).astype(BF16))
    sched = dict(nlo=nlo.tolist(), nhi=nhi.tolist(),
                 lo_off=lo_off.tolist(), hi_off=hi_off.tolist(),
                 TL=TL, TH=TH, TLp=TLp, THp=THp, TB=TB,
                 blk_grp=blk_grp.tolist())
    return idx_ws, ohs, ohTs, sched


def kernel(**ins):
    global LAST_RESULT
    import concourse.bass as bass
    import concourse.tile as tile
    from concourse import bacc, mybir
    from concourse.bass_utils import run_bass_kernel_spmd

    FP = mybir.dt.float32
    BF = mybir.dt.bfloat16
    I16 = mybir.dt.int16
    AL = mybir.AluOpType
    AF = mybir.ActivationFunctionType

    f = _fold_weights(ins)
    idx0, oh0, ohT0, sc0 = _prep_edges(np.asarray(ins["edge_ab"]))
    idx1, oh1, ohT1, sc1 = _prep_edges(np.asarray(ins["edge_ba"]))
    scheds = [sc0, sc1]
    TBs = [sc0["TB"], sc1["TB"]]

    xa = np.asarray(ins["x_a"], np.float32)
    xb = np.asarray(ins["x_b"], np.float32)
    DA, DB = xa.shape[1], xb.shape[1]
    # transposed, zero-padded, ones row appended (bias fold), bf16
    xaT = np.zeros((DA + 1, NPAD), np.float32)
    xaT[:DA, :40000] = xa.T
    xaT[DA, :] = 1.0
    xbT = np.zeros((DB + 1, NPAD), np.float32)
    xbT[:DB, :40000] = xb.T
    xbT[DB, :] = 1.0
    xaT = xaT.astype(BF16)
    xbT = xbT.astype(BF16)

    nc = bacc.Bacc("TRN2", target_bir_lowering=False, debug=False, num_devices=NCORES)

    # ---- DRAM tensors ----
    t_xasT = nc.dram_tensor("xasT", [DA + 1, SHARD], BF, kind="ExternalInput").ap()
    t_xbsT = nc.dram_tensor("xbsT", [DB + 1, SHARD], BF, kind="ExternalInput").ap()
    wnames = ["Wina", "Winb"]
    for l in range(L):
        for t in range(2):
            wnames += [f"Wkvq{l}{t}", f"Bq{l}{t}",
                       f"Wal{l}{t}", f"Bal{l}{t}", f"Bv{l}{t}"]
    t_w = {n: nc.dram_tensor(n, list(f[n].shape), BF, kind="ExternalInput").ap()
           for n in wnames}
    t_idx = [nc.dram_tensor(f"idx{r}", [P, TBs[r] * 8], I16, kind="ExternalInput").ap()
             for r in range(2)]
    t_oh = [nc.dram_tensor(f"oh{r}", [P, TBs[r] * P], BF, kind="ExternalInput").ap()
            for r in range(2)]
    t_ohT = [nc.dram_tensor(f"ohT{r}", [P, TBs[r] * P], BF, kind="ExternalInput").ap()
             for r in range(2)]

    # K|V tables per (src type, layer): AllGather output, viewed flat for gathers
    t_tab = [[nc.dram_tensor(f"tab{t}{l}", [NCORES, SHARD, 2 * C], BF,
                             addr_space="Shared") for l in range(L)]
             for t in range(2)]
    t_agsrc = [[nc.dram_tensor(f"agsrc{t}{l}", [SHARD, 2 * C], BF)
                for l in range(L)] for t in range(2)]
    t_out = [nc.dram_tensor(f"out{t}", [SHARD, C], FP, kind="ExternalOutput").ap()
             for t in range(2)]

    with tile.TileContext(nc) as tc:
        cpool_cm = tc.tile_pool(name="const", bufs=1)
        cpool = cpool_cm.__enter__()
        ident = cpool.tile([P, P], BF)
        from concourse.masks import make_identity
        make_identity(nc, ident[:])
        w_sb = {}
        for n in wnames:
            w_sb[n] = cpool.tile(list(f[n].shape), BF, name=n, tag=n)
            nc.sync.dma_start(out=w_sb[n][:], in_=t_w[n][:])
        idx_sb = []
        for r in range(2):
            it = cpool.tile([P, TBs[r] * 8], I16, name=f"idxsb{r}", tag=f"idxsb{r}")
            nc.sync.dma_start(out=it[:], in_=t_idx[r][:])
            idx_sb.append(it)
        # persistent per-shard state
        q_sb = [cpool.tile([P, NGRP, C], BF, name=f"qsb{t}", tag=f"qsb{t}")
                for t in range(2)]
        x_sb = [[cpool.tile([P, NGRP, C], BF, name=f"xsb{t}{l}", tag=f"xsb{t}{l}")
                 for l in range(2)] for t in range(2)]
        acc_sb = [cpool.tile([P, NGRP, 132], FP, name=f"accsb{t}", tag=f"accsb{t}")
                  for t in range(2)]
        xsT_sb = {}
        for t, (ap_, din) in enumerate([(t_xasT, DA + 1), (t_xbsT, DB + 1)]):
            xt = cpool.tile([din, SHARD], BF, name=f"xsT{t}", tag=f"xsT{t}")
            nc.sync.dma_start(out=xt[:], in_=ap_[:])
            xsT_sb[t] = xt

        def ag(t, l):
            if os.environ.get("SKIP_AG"):
                for k in range(NCORES):
                    nc.sync.dma_start(out=t_tab[t][l].ap()[k, :, :],
                                      in_=t_agsrc[t][l].ap()[:])
            else:
                nc.gpsimd.collective_compute(
                    "AllGather", mybir.AluOpType.bypass,
                    replica_groups=[list(range(NCORES))],
                    ins=[t_agsrc[t][l].ap()[:]], outs=[t_tab[t][l].ap()[:]],
                )

        # ---------- phase 1: layer-0 own-shard projections ----------
        def phase1(t):
            Win = "Wina" if t == 0 else "Winb"
            din = (DA if t == 0 else DB) + 1
            with (
                tc.tile_pool(name=f"p1s{t}", bufs=3) as sp,
                tc.tile_pool(name=f"p1p{t}", bufs=2, space="PSUM") as pp,
            ):
                for g in range(NGRP):
                    ps0 = pp.tile([P, C], FP, space="PSUM", tag="c1")
                    nc.tensor.matmul(out=ps0[:], lhsT=xsT_sb[t][:, g * P:(g + 1) * P],
                                     rhs=w_sb[Win][:], start=True, stop=True)
                    # relu + cast into resident x0
                    nc.scalar.activation(out=x_sb[t][0][:, g, :], in_=ps0[:], func=AF.Relu)
                    pst = pp.tile([P, P], BF, space="PSUM", tag="pst")
                    nc.tensor.transpose(out=pst[:], in_=x_sb[t][0][:, g, :], identity=ident[:])
                    x0T = sp.tile([P, P], BF, tag="x0T")
                    nc.scalar.activation(out=x0T[:], in_=pst[:], func=AF.Copy)
                    pkv = pp.tile([P, 3 * C], FP, space="PSUM", tag="c2")
                    nc.tensor.matmul(out=pkv[:], lhsT=x0T[:], rhs=w_sb[f"Wkvq0{t}"][:],
                                     start=True, stop=True)
                    kvt = sp.tile([P, 2 * C], BF, tag="kvt")
                    nc.scalar.activation(out=kvt[:], in_=pkv[:, 0:2 * C], func=AF.Copy)
                    nc.sync.dma_start(out=t_agsrc[t][0].ap()[g * P:(g + 1) * P, :], in_=kvt[:])
                    nc.vector.tensor_tensor(out=q_sb[t][:, g, :], in0=pkv[:, 2 * C:3 * C],
                                            in1=w_sb[f"Bq0{t}"][:], op=AL.add)

        # ---------- attention ----------
        gpool = bpool = aps = accp = None

        def attention(r, l, mid_cb=None):
            """relation r: src type r, dst type 1-r; fills acc_sb[1-r]."""
            sc = scheds[r]
            td = 1 - r
            tabflat = t_tab[r][l].ap().rearrange("k n c -> (k n) c")
            qt = q_sb[td]
            idxt = idx_sb[r]
            blk_grp = sc["blk_grp"]
            if True:
                chunk_no = 0
                for region in range(2):
                    r0 = 0 if region == 0 else sc["TLp"]
                    r1 = sc["TLp"] if region == 0 else sc["TB"]
                    nblk_reg = r1 - r0
                    if nblk_reg == 0:
                        continue
                    in_ap = tabflat[0:LO_LIM, :] if region == 0 else tabflat[LO_LIM:NPAD, :]
                    accps = None
                    cur_grp = -1
                    for c0 in range(r0, r1, CHUNK_BLK):
                        chunk_no += 1
                        if chunk_no == 4 and mid_cb is not None:
                            mid_cb()
                        n = min(CHUNK_BLK, r1 - c0)
                        gt = gpool.tile([P, CHUNK_BLK, 2 * C], BF, tag="kvchunk")
                        if os.environ.get("SKIP_GATHER"):
                            nc.vector.memset(gt[:, 0:n, :], 1.0)
                        else:
                            nc.gpsimd.dma_gather(
                                out_ap=gt[:, 0:n, :], in_ap=in_ap,
                                idxs_ap=idxt[:, c0 * 8:(c0 + n) * 8],
                                num_idxs=n * P, num_idxs_reg=n * P,
                                elem_size=2 * C, single_packet=False,
                            )
                        oht_c = gpool.tile([P, CHUNK_BLK, P], BF, tag="ohTchunk")
                        nc.sync.dma_start(
                            out=oht_c[:, 0:n, :].rearrange("p a b -> p (a b)"),
                            in_=t_ohT[r][:, c0 * P:(c0 + n) * P])
                        oh_c = gpool.tile([P, CHUNK_BLK, P], BF, tag="ohchunk")
                        nc.sync.dma_start(
                            out=oh_c[:, 0:n, :].rearrange("p a b -> p (a b)"),
                            in_=t_oh[r][:, c0 * P:(c0 + n) * P])
                        for b0 in range(0, n, B):
                            nb = min(B, n - b0)
                            qg_ps = aps.tile([P, B, C], FP, space="PSUM", tag="qg")
                            for i in range(nb):
                                g = blk_grp[c0 + b0 + i]
                                nc.tensor.matmul(out=qg_ps[:, i, :],
                                                 lhsT=oht_c[:, b0 + i, :],
                                                 rhs=qt[:, g, :], start=True, stop=True)
                            qg = bpool.tile([P, B, C], BF, tag="qg_sb")
                            nc.scalar.activation(out=qg[:, 0:nb, :], in_=qg_ps[:, 0:nb, :],
                                                 func=AF.Copy)
                            lp = bpool.tile([P, B, C], BF, tag="lp")
                            nc.vector.tensor_tensor(out=lp[:, 0:nb, :], in0=qg[:, 0:nb, :],
                                                    in1=gt[:, b0:b0 + nb, 0:C], op=AL.mult)
                            z = bpool.tile([P, B * H], FP, tag="z")
                            nc.vector.tensor_reduce(
                                out=z[:, 0:nb * H],
                                in_=lp[:, 0:nb, :].rearrange("p b (h d) -> p (b h) d", h=H),
                                axis=mybir.AxisListType.X, op=AL.add)
                            ze = bpool.tile([P, B * H], BF, tag="ze")
                            nc.scalar.activation(out=ze[:, 0:nb * H], in_=z[:, 0:nb * H],
                                                 func=AF.Exp)
                            zx = bpool.tile([P, B, C], BF, tag="zx")
                            nc.scalar.activation(
                                out=zx[:, 0:nb, :].rearrange("p b (h d) -> p b h d", h=H),
                                in_=ze[:, 0:nb * H].rearrange("p (b h) -> p b h ()", h=H)
                                    .to_broadcast([P, nb, H, D]),
                                func=AF.Copy)
                            wz = bpool.tile([P, B, 132], BF, tag="wz")
                            nc.vector.tensor_tensor(out=wz[:, 0:nb, 0:C],
                                                    in0=gt[:, b0:b0 + nb, C:2 * C],
                                                    in1=zx[:, 0:nb, :], op=AL.mult)
                            nc.scalar.activation(
                                out=wz[:, 0:nb, C:C + H],
                                in_=ze[:, 0:nb * H].rearrange("p (b h) -> p b h", h=H),
                                func=AF.Copy)
                            for i in range(nb):
                                blk = c0 + b0 + i
                                g = blk_grp[blk]
                                if g != cur_grp:
                                    accps = accp.tile([P, 132], FP, space="PSUM", tag="acc")
                                    cur_grp = g
                                off = sc["lo_off"][g] if region == 0 else sc["hi_off"][g]
                                cnt = sc["nlo"][g] if region == 0 else sc["nhi"][g]
                                end = off + cnt
                                if g == NGRP - 1:
                                    end = r1    # pads attach to last group
                                first = blk == off
                                last = blk == end - 1
                                nc.tensor.matmul(out=accps[:], lhsT=oh_c[:, b0 + i, :],
                                                 rhs=wz[:, i, :], start=first, stop=last)
                                if last:
                                    if region == 0:
                                        nc.scalar.activation(out=acc_sb[td][:, g, :],
                                                             in_=accps[:], func=AF.Copy)
                                    else:
                                        nc.vector.tensor_tensor(
                                            out=acc_sb[td][:, g, :], in0=accps[:],
                                            in1=acc_sb[td][:, g, :], op=AL.add)

        # ---------- alin ----------
        def alin(t, l):
            """a_lin + skip for dst type t, layer l; reads acc_sb[t]. For l=0
            also produces layer-1 q, resident x1, and the layer-1 K|V shard."""
            with (
                tc.tile_pool(name=f"al{t}{l}", bufs=3) as sp,
                tc.tile_pool(name=f"alp{t}{l}", bufs=1, space="PSUM") as pp,
            ):
                for g in range(NGRP):
                    den = sp.tile([P, H], FP, tag="den")
                    nc.vector.tensor_scalar(out=den[:], in0=acc_sb[t][:, g, C:C + H],
                                            scalar1=1e-16, scalar2=None, op0=AL.add)
                    rec = sp.tile([P, H], FP, tag="rec")
                    nc.vector.reciprocal(rec[:], den[:])
                    at = sp.tile([P, C], BF, tag="at")
                    nc.vector.tensor_tensor(
                        out=at[:], in0=acc_sb[t][:, g, 0:C],
                        in1=rec[:].rearrange("p (h o) -> p h o", o=1).to_broadcast([P, H, D]),
                        op=AL.mult)
                    at2 = sp.tile([P, C], BF, tag="at2")
                    nc.vector.tensor_tensor(out=at2[:], in0=at[:], in1=w_sb[f"Bv{l}{t}"][:],
                                            op=AL.add)
                    gl = sp.tile([P, C], BF, tag="gl")
                    nc.scalar.activation(out=gl[:], in_=at2[:], func=AF.Gelu)
                    pst = pp.tile([P, P], BF, space="PSUM", tag="trans")
                    nc.tensor.transpose(out=pst[:], in_=gl[:], identity=ident[:])
                    glT = sp.tile([P, P], BF, tag="glT")
                    nc.scalar.activation(out=glT[:], in_=pst[:], func=AF.Copy)
                    pso = pp.tile([P, C], FP, space="PSUM", tag="c1")
                    nc.tensor.matmul(out=pso[:], lhsT=glT[:], rhs=w_sb[f"Wal{l}{t}"][:],
                                     start=True, stop=True)
                    o1 = sp.tile([P, C], BF, tag="o1")
                    nc.vector.tensor_tensor(out=o1[:], in0=pso[:], in1=w_sb[f"Bal{l}{t}"][:],
                                            op=AL.add)
                    o2 = sp.tile([P, C], BF, tag="o2")
                    nc.scalar.activation(out=o2[:], in_=x_sb[t][l][:, g, :],
                                         func=AF.Copy, scale=float(f[f"oms{l}{t}"]))
                    if l == 0:
                        nw = x_sb[t][1][:, g, :]
                        nc.vector.tensor_tensor(out=nw, in0=o1[:], in1=o2[:], op=AL.add)
                        pst2 = pp.tile([P, P], BF, space="PSUM", tag="trans2")
                        nc.tensor.transpose(out=pst2[:], in_=nw, identity=ident[:])
                        nwT = sp.tile([P, P], BF, tag="nwT")
                        nc.scalar.activation(out=nwT[:], in_=pst2[:], func=AF.Copy)
                        pkv = pp.tile([P, 3 * C], FP, space="PSUM", tag="c2")
                        nc.tensor.matmul(out=pkv[:], lhsT=nwT[:], rhs=w_sb[f"Wkvq1{t}"][:],
                                         start=True, stop=True)
                        kvt = sp.tile([P, 2 * C], BF, tag="alkvt")
                        nc.scalar.activation(out=kvt[:], in_=pkv[:, 0:2 * C], func=AF.Copy)
                        nc.sync.dma_start(out=t_agsrc[t][1].ap()[g * P:(g + 1) * P, :],
                                          in_=kvt[:])
                        nc.vector.tensor_tensor(out=q_sb[t][:, g, :], in0=pkv[:, 2 * C:3 * C],
                                                in1=w_sb[f"Bq1{t}"][:], op=AL.add)
                    else:
                        nw = sp.tile([P, C], FP, tag="nwf")
                        nc.vector.tensor_tensor(out=nw[:], in0=o1[:], in1=o2[:], op=AL.add)
                        nc.sync.dma_start(out=t_out[t][g * P:(g + 1) * P, :], in_=nw[:])

        # ---------- schedule ----------
        phase1(0)
        ag(0, 0)
        phase1(1)
        ag(1, 0)
        gpool_cm = tc.tile_pool(name="gat", bufs=3)
        gpool = gpool_cm.__enter__()
        bpool_cm = tc.tile_pool(name="bat", bufs=3)
        bpool = bpool_cm.__enter__()
        aps_cm = tc.tile_pool(name="aps", bufs=2, space="PSUM")
        aps = aps_cm.__enter__()
        accp_cm = tc.tile_pool(name="accp", bufs=2, space="PSUM")
        accp = accp_cm.__enter__()
        attention(0, 0)     # dst b, table a/l0
        alin(1, 0)          # x1_b, kv_b_l1 shard
        attention(1, 0, mid_cb=lambda: ag(1, 1))   # dst a, table b/l0
        alin(0, 0)          # x1_a, kv_a_l1 shard
        attention(1, 1, mid_cb=lambda: ag(0, 1))   # dst a, table b/l1
        alin(0, 1)          # out a
        attention(0, 1)     # dst b, table a/l1
        alin(1, 1)          # out b
        accp_cm.__exit__(None, None, None)
        aps_cm.__exit__(None, None, None)
        bpool_cm.__exit__(None, None, None)
        gpool_cm.__exit__(None, None, None)
        cpool_cm.__exit__(None, None, None)

    nc.compile()

    in_maps = []
    for c in range(NCORES):
        m = {"xasT": np.ascontiguousarray(xaT[:, c * SHARD:(c + 1) * SHARD]),
             "xbsT": np.ascontiguousarray(xbT[:, c * SHARD:(c + 1) * SHARD]),
             "idx0": idx0[c], "oh0": oh0[c], "ohT0": ohT0[c],
             "idx1": idx1[c], "oh1": oh1[c], "ohT1": ohT1[c]}
        for n in wnames:
            m[n] = np.ascontiguousarray(f[n])
        in_maps.append(m)

    res = run_bass_kernel_spmd(
        nc, in_maps, core_ids=list(range(NCORES)),
        trace=bool(os.environ.get("BASS_TRACE")),
    )
    LAST_RESULT = res
    outa = np.concatenate([res.results[c]["out0"] for c in range(NCORES)])[:40000]
    outb = np.concatenate([res.results[c]["out1"] for c in range(NCORES)])[:40000]
    return outa, outb
